# revision 5
# baseline (speedup 1.0000x reference)
"""Trainium2 Bass kernel for nn_ABNet_U (multi-branch MLP + CBF-QP head).

Data-parallel over batch: 16384 rows -> 8 NeuronCores x 2048 rows.
Weights replicated, host-prepped into K-major bf16 layouts; all GEMMs run
on the TensorEngine with fp32 PSUM accumulation and fused bias+activation
eviction on the Scalar/Vector engines.  The trig/QP tail runs in fp32 on
the Vector/Scalar engines with batch on partitions, emitted early so it
overlaps the GEMM phase.
"""

import sys

sys.path.insert(0, "/opt/trn_rl_repo")

import numpy as np
import ml_dtypes

import concourse.bass as bass
import concourse.mybir as mybir
import concourse.tile as tile
from concourse import bacc
from concourse.bass_utils import run_bass_kernel_spmd
from concourse.masks import make_identity

N_CORES = 8
B_GLOBAL = 16384
B = B_GLOBAL // N_CORES  # 2048 rows per core
P = 128
CH = B // P              # 16 batch chunks of 128 (tail layout)
NF = 512                 # matmul free-dim chunk
NB = B // NF             # 4 free chunks
HEADS = 10

AF = mybir.ActivationFunctionType
ALU = mybir.AluOpType
AX = mybir.AxisListType
F32 = mybir.dt.float32
BF16 = mybir.dt.bfloat16
I32 = mybir.dt.int32

TWO_PI = float(2.0 * np.pi)
HALF_PI = float(0.5 * np.pi)

_CACHED_NC = None


def _build():
    nc = bacc.Bacc(
        "TRN2",
        target_bir_lowering=False,
        debug=False,
        enable_asserts=False,
        num_devices=N_CORES,
    )

    def din(name, shape, dt=F32):
        return nc.dram_tensor(name, list(shape), dt, kind="ExternalInput").ap()

    xt = din("xt", (4, B), BF16)            # x shard, transposed, bf16
    xn = din("xn", (P, CH, 4))              # x shard, [p, chunk, feat] fp32
    w1 = din("w1", (4, 2048), BF16)
    w2 = din("w2", (P, 16, 16, P), BF16)    # [p, mt, kt, mc]
    w3 = din("w3", (P, 16, 16, P), BF16)
    w41 = din("w41", (P, 8, 8, P), BF16)
    w42 = din("w42", (P, 8, 8, P), BF16)
    w51 = din("w51", (P, 8, 20), BF16)      # [p, kt, m]
    w52 = din("w52", (P, 8, 11), BF16)
    b1 = din("b1", (P, 16))
    b2 = din("b2", (P, 16))
    b3 = din("b3", (P, 16))
    b41 = din("b41", (P, 8))
    b42 = din("b42", (P, 8))
    b51 = din("b51", (20,))
    b52 = din("b52", (11,))
    stdb = din("stdb", (P, 4))
    meanb = din("meanb", (P, 4))
    mlb = din("mlb", (P, 2))
    islb = din("islb", (P, 2))
    wtv = din("wtv", (10,))
    out = nc.dram_tensor("out", [P, CH, 2], F32, kind="ExternalOutput").ap()

    with tile.TileContext(nc) as tc:
        from contextlib import ExitStack

        with ExitStack() as ctx:
            const = ctx.enter_context(tc.tile_pool(name="const", bufs=1))
            wpool = ctx.enter_context(tc.tile_pool(name="wpool", bufs=3))
            hpool = ctx.enter_context(tc.tile_pool(name="hpool", bufs=2))
            psum = ctx.enter_context(tc.tile_pool(name="psum", bufs=6, space="PSUM"))
            pstr = ctx.enter_context(tc.tile_pool(name="pstr", bufs=2, space="PSUM"))
            tp = ctx.enter_context(tc.tile_pool(name="tp", bufs=1))

            # ---- L1-critical loads first: keep the PE fed from t=0 ----
            b1t = const.tile([P, 16], F32, tag="b1")
            nc.sync.dma_start(b1t[:], b1)
            xtb = const.tile([P, B], BF16, tag="xtb")
            nc.vector.memset(xtb[:], 0.0)
            nc.sync.dma_start(xtb[:4, :], xt)
            w1tb = const.tile([P, 2048], BF16, tag="w1tb")
            nc.vector.memset(w1tb[:], 0.0)
            nc.sync.dma_start(w1tb[:4, :], w1)

            # prefetch: L2's first weight column gates the L1->L2 handoff;
            # the tiny L5 weight tiles ride the otherwise-idle gpsimd queue.
            wcol2_0 = wpool.tile([P, 16, P], BF16, tag="wcol", name="wcol2_0")
            nc.sync.dma_start(wcol2_0[:], w2[:, 0])
            w51c = const.tile([P, 8, 20], BF16, tag="w51c")
            nc.gpsimd.dma_start(w51c[:], w51)
            w52c = const.tile([P, 8, 11], BF16, tag="w52c")
            nc.gpsimd.dma_start(w52c[:], w52)

            # ---- L1: h1 = relu(W1 @ x^T + b1), K=4 zero-padded to 128 ----
            # One matmul per eviction, so evictions bound this layer: split
            # them across the Scalar and Vector engines.
            h1 = hpool.tile([P, 16, B], BF16, tag="act")
            flip = 0
            for m in range(16):
                for n in range(NB):
                    ps = psum.tile([P, NF], F32, tag="mm")
                    nc.tensor.matmul(
                        ps[:],
                        w1tb[:, m * P : (m + 1) * P],
                        xtb[:, n * NF : (n + 1) * NF],
                        start=True,
                        stop=True,
                    )
                    dst = h1[:, m, n * NF : (n + 1) * NF]
                    if flip % 2 == 0:
                        nc.scalar.activation(
                            dst, ps[:], AF.Relu, bias=b1t[:, m : m + 1]
                        )
                    else:
                        nc.vector.tensor_scalar(
                            dst, ps[:], b1t[:, m : m + 1], 0.0,
                            op0=ALU.add, op1=ALU.max,
                        )
                    flip += 1

            # ---- remaining constants (emitted after L1 so they never gate it)
            b2t = const.tile([P, 16], F32, tag="b2")
            nc.sync.dma_start(b2t[:], b2)
            b3t = const.tile([P, 16], F32, tag="b3")
            nc.sync.dma_start(b3t[:], b3)
            b41t = const.tile([P, 8], F32, tag="b41")
            nc.sync.dma_start(b41t[:], b41)
            b42t = const.tile([P, 8], F32, tag="b42")
            nc.sync.dma_start(b42t[:], b42)
            b51t = const.tile([20, 1], F32, tag="b51")
            nc.sync.dma_start(b51t[:], b51[:, None])
            b52t = const.tile([11, 1], F32, tag="b52")
            nc.sync.dma_start(b52t[:], b52[:, None])
            stdt = const.tile([P, 4], F32, tag="stdt")
            nc.sync.dma_start(stdt[:], stdb)
            meant = const.tile([P, 4], F32, tag="meant")
            nc.sync.dma_start(meant[:], meanb)
            mlt = const.tile([P, 2], F32, tag="mlt")
            nc.sync.dma_start(mlt[:], mlb)
            islt = const.tile([P, 2], F32, tag="islt")
            nc.sync.dma_start(islt[:], islb)
            halfpi = const.tile([P, 1], F32, tag="halfpi")
            nc.vector.memset(halfpi[:], HALF_PI)
            ident = const.tile([P, P], F32)
            make_identity(nc, ident[:])

            # softmax(wt) DVE chain (PE broadcast deferred until after L4)
            wtt = const.tile([1, 10], F32, tag="wtt")
            nc.sync.dma_start(wtt[:], wtv[None, :])
            mx = const.tile([1, 1], F32, tag="mx")
            nc.vector.reduce_max(mx[:, 0:1], wtt[:], axis=AX.X)
            nm = const.tile([1, 1], F32, tag="nm")
            nc.vector.tensor_scalar_mul(nm[:], mx[:], -1.0)
            ex = const.tile([1, 10], F32, tag="ex")
            nc.scalar.activation(ex[:], wtt[:], AF.Exp, bias=nm[:])
            sm = const.tile([1, 1], F32, tag="sm")
            nc.vector.reduce_sum(sm[:, 0:1], ex[:], axis=AX.X)
            inv = const.tile([1, 1], F32, tag="inv")
            nc.vector.reciprocal(inv[:], sm[:])
            wv10 = const.tile([1, 10], F32, tag="wv10")
            nc.vector.tensor_scalar_mul(wv10[:], ex[:], inv[:])
            wvp = const.tile([32, 32], F32, tag="wvp")
            nc.vector.memset(wvp[:], 0.0)
            nc.vector.tensor_copy(
                wvp[0:1, 0:20].rearrange("p (h c) -> p h c", c=2),
                wv10[:, :, None].to_broadcast([1, 10, 2]),
            )
            onesp = const.tile([32, P], F32, tag="onesp")
            nc.vector.memset(onesp[:], 0.0)
            nc.vector.memset(onesp[0:1, :], 1.0)

            # combined x51/x52 head tile (rows 0..19 = x51, 20..30 = x52)
            x5cat = tp.tile([64, B], F32, tag="x5cat")
            nc.vector.memset(x5cat[:], 0.0)

            # ---- tail part 1: geometry from x only — emitted now so the
            # Vector engine computes it underneath the L2..L5 GEMMs.
            def t3(tag, d=1):
                return tp.tile([P, CH, d], F32, tag=tag, name=tag)

            xnt = t3("xnt", 4)
            nc.sync.dma_start(xnt[:], xn)
            x0 = t3("x0", 4)
            nc.vector.tensor_mul(
                x0[:], xnt[:], stdt[:, None, :].to_broadcast([P, CH, 4])
            )
            nc.vector.tensor_add(
                x0[:], x0[:], meant[:, None, :].to_broadcast([P, CH, 4])
            )

            th = x0[:, :, 0::2]   # [P, CH, 2] angles
            wv_ = x0[:, :, 1::2]  # [P, CH, 2] angular velocities

            # range-reduce th -> rs in [-pi, pi]:  rs = th - 2pi*rint(th/2pi)
            q = t3("q", 2)
            qi = tp.tile([P, CH, 2], I32, tag="qi")
            qr = t3("qr", 2)
            rs = t3("rs", 2)
            nc.vector.tensor_scalar_mul(q[:], th, 1.0 / TWO_PI)
            nc.vector.tensor_copy(qi[:], q[:])
            nc.vector.tensor_copy(qr[:], qi[:])
            nc.vector.scalar_tensor_tensor(
                rs[:], in0=qr[:], scalar=-TWO_PI, in1=th,
                op0=ALU.mult, op1=ALU.add,
            )
            # range-reduce th + pi/2 -> rc (for cos)
            qc = t3("qc", 2)
            qci = tp.tile([P, CH, 2], I32, tag="qci")
            qcr = t3("qcr", 2)
            rc = t3("rc", 2)
            nc.vector.tensor_scalar(
                qc[:], th, 1.0 / TWO_PI, 0.25, op0=ALU.mult, op1=ALU.add
            )
            nc.vector.tensor_copy(qci[:], qc[:])
            nc.vector.tensor_copy(qcr[:], qci[:])
            nc.vector.scalar_tensor_tensor(
                rc[:], in0=qcr[:], scalar=-TWO_PI, in1=th,
                op0=ALU.mult, op1=ALU.add,
            )
            nc.vector.tensor_scalar_add(rc[:], rc[:], HALF_PI)

            sn = t3("sn", 2)
            cs = t3("cs", 2)
            nc.scalar.activation(sn[:], rs[:], AF.Sin)
            nc.scalar.activation(cs[:], rc[:], AF.Sin)

            s1, s2 = sn[:, :, 0:1], sn[:, :, 1:2]
            c1, c2 = cs[:, :, 0:1], cs[:, :, 1:2]
            w1v, w2v = wv_[:, :, 0:1], wv_[:, :, 1:2]

            px = t3("px")
            nc.vector.tensor_add(px[:], c1, c2)
            nc.vector.tensor_scalar_mul(px[:], px[:], 3.0)
            py = t3("py")
            nc.vector.tensor_add(py[:], s1, s2)
            nc.vector.tensor_scalar(py[:], py[:], 3.0, -7.0, op0=ALU.mult, op1=ALU.add)

            s1w = t3("s1w")
            nc.vector.tensor_mul(s1w[:], s1, w1v)
            s2w = t3("s2w")
            nc.vector.tensor_mul(s2w[:], s2, w2v)
            vx = t3("vx")
            nc.vector.tensor_add(vx[:], s1w[:], s2w[:])
            nc.vector.tensor_scalar_mul(vx[:], vx[:], -3.0)
            c1w = t3("c1w")
            nc.vector.tensor_mul(c1w[:], c1, w1v)
            c2w = t3("c2w")
            nc.vector.tensor_mul(c2w[:], c2, w2v)
            vy = t3("vy")
            nc.vector.tensor_add(vy[:], c1w[:], c2w[:])
            nc.vector.tensor_scalar_mul(vy[:], vy[:], 3.0)

            pxx = t3("pxx")
            nc.vector.tensor_mul(pxx[:], px[:], px[:])
            pyy = t3("pyy")
            nc.vector.tensor_mul(pyy[:], py[:], py[:])
            # barrier scaled by 16 = alpha*beta scale (4*sigmoid each)
            barrier = t3("barrier")
            nc.vector.tensor_add(barrier[:], pxx[:], pyy[:])
            nc.vector.tensor_scalar(
                barrier[:], barrier[:], 16.0, -256.0, op0=ALU.mult, op1=ALU.add
            )

            pv1 = t3("pv1")
            nc.vector.tensor_mul(pv1[:], px[:], vx[:])
            pv2 = t3("pv2")
            nc.vector.tensor_mul(pv2[:], py[:], vy[:])
            b_dot = t3("b_dot")
            nc.vector.tensor_add(b_dot[:], pv1[:], pv2[:])
            # 2 (from b_dot) * 4 (alpha+beta sigmoid scale)
            nc.vector.tensor_scalar_mul(b_dot[:], b_dot[:], 8.0)

            w1sq = t3("w1sq")
            nc.vector.tensor_mul(w1sq[:], w1v, w1v)
            w2sq = t3("w2sq")
            nc.vector.tensor_mul(w2sq[:], w2v, w2v)
            ca = t3("ca")
            nc.vector.tensor_mul(ca[:], c1, w1sq[:])
            cb = t3("cb")
            nc.vector.tensor_mul(cb[:], c2, w2sq[:])
            nc.vector.tensor_add(ca[:], ca[:], cb[:])   # c1*w1^2 + c2*w2^2
            sa = t3("sa")
            nc.vector.tensor_mul(sa[:], s1, w1sq[:])
            sb = t3("sb")
            nc.vector.tensor_mul(sb[:], s2, w2sq[:])
            nc.vector.tensor_add(sa[:], sa[:], sb[:])   # s1*w1^2 + s2*w2^2

            vxx = t3("vxx")
            nc.vector.tensor_mul(vxx[:], vx[:], vx[:])
            vyy = t3("vyy")
            nc.vector.tensor_mul(vyy[:], vy[:], vy[:])
            vsum = t3("vsum")
            nc.vector.tensor_add(vsum[:], vxx[:], vyy[:])
            nc.vector.tensor_scalar_mul(vsum[:], vsum[:], 2.0)  # 2vx^2+2vy^2

            pca = t3("pca")
            nc.vector.tensor_mul(pca[:], px[:], ca[:])
            psa = t3("psa")
            nc.vector.tensor_mul(psa[:], py[:], sa[:])
            nc.vector.tensor_add(pca[:], pca[:], psa[:])
            lf2b = t3("lf2b")
            nc.vector.scalar_tensor_tensor(
                lf2b[:], in0=pca[:], scalar=-6.0, in1=vsum[:],
                op0=ALU.mult, op1=ALU.add,
            )  # Lf2b = 2(vx^2+vy^2) - 6*(px*ca + py*sa)

            g1 = t3("g1")
            m1 = t3("m1")
            nc.vector.tensor_mul(m1[:], py[:], c1)
            m2 = t3("m2")
            nc.vector.tensor_mul(m2[:], px[:], s1)
            nc.vector.tensor_sub(g1[:], m1[:], m2[:])
            nc.vector.tensor_scalar_mul(g1[:], g1[:], 6.0)
            g2 = t3("g2")
            nc.vector.tensor_mul(m1[:], py[:], c2)
            nc.vector.tensor_mul(m2[:], px[:], s2)
            nc.vector.tensor_sub(g2[:], m1[:], m2[:])
            nc.vector.tensor_scalar_mul(g2[:], g2[:], 6.0)

            gdot = t3("gdot")
            g1sq = t3("g1sq")
            nc.vector.tensor_mul(g1sq[:], g1[:], g1[:])
            g2sq = t3("g2sq")
            nc.vector.tensor_mul(g2sq[:], g2[:], g2[:])
            nc.vector.tensor_add(gdot[:], g1sq[:], g2sq[:])
            igdot = t3("igdot")
            nc.vector.reciprocal(igdot[:], gdot[:])

            # ---- generic streamed GEMM layer ----
            def mlp_layer(wdram, KT, MT, MD, hin, kin_base, btile, evict,
                          prefetched=None):
                for m in range(MT):
                    mp = min(P, MD - m * P)
                    if prefetched is not None and m in prefetched:
                        wcol = prefetched[m]
                    else:
                        wcol = wpool.tile([P, KT, mp], BF16, tag="wcol")
                        if len(wdram.shape) == 4:
                            nc.sync.dma_start(wcol[:], wdram[:, m])
                        else:
                            nc.sync.dma_start(wcol[:], wdram)
                    for n in range(NB):
                        ps = psum.tile([P, NF], F32, tag="mm")
                        for k in range(KT):
                            nc.tensor.matmul(
                                ps[:mp],
                                wcol[:, k, :],
                                hin[:, kin_base + k, n * NF : (n + 1) * NF],
                                start=(k == 0),
                                stop=(k == KT - 1),
                            )
                        evict(m, n, ps[:mp])

            # ---- L2 / L3 / L4 ----
            h2 = hpool.tile([P, 16, B], BF16, tag="act")

            def ev_h(hout, btile, m_off=0):
                def _e(m, n, ps):
                    nc.scalar.activation(
                        hout[:, m_off + m, n * NF : (n + 1) * NF], ps, AF.Relu,
                        bias=btile[:, m : m + 1],
                    )
                return _e

            mlp_layer(w2, 16, 16, 2048, h1, 0, b2t, ev_h(h2, b2t), prefetched={0: wcol2_0})

            h3 = hpool.tile([P, 16, B], BF16, tag="act")
            mlp_layer(w3, 16, 16, 2048, h2, 0, b3t, ev_h(h3, b3t))

            h4 = hpool.tile([P, 16, B], BF16, tag="act")
            mlp_layer(w41, 8, 8, 1024, h3, 0, b41t, ev_h(h4, b41t, 0))
            mlp_layer(w42, 8, 8, 1024, h3, 8, b42t, ev_h(h4, b42t, 8))

            # ---- L5 into the combined head tile ----
            def ev_51(m, n, ps):
                nc.vector.tensor_scalar_add(
                    x5cat[:20, n * NF : (n + 1) * NF], ps, b51t[:]
                )

            def ev_52(m, n, ps):
                nc.scalar.activation(
                    x5cat[32:43, n * NF : (n + 1) * NF], ps, AF.Sigmoid,
                    bias=b52t[:],
                )

            mlp_layer(w51, 8, 1, 20, h4, 0, b51t, ev_51, prefetched={0: w51c})
            mlp_layer(w52, 8, 1, 11, h4, 8, b52t, ev_52, prefetched={0: w52c})

            # wv broadcast to all partitions (PE hits this after L4/L5)
            pwv = pstr.tile([P, 64], F32, tag="tr")
            nc.tensor.matmul(pwv[:, :32], onesp[:], wvp[:], start=True, stop=True)
            wv20 = const.tile([P, 20], F32, tag="wv20")
            nc.vector.tensor_copy(wv20[:], pwv[:, :20])

            # ---- transpose the combined head tile: 16 chunks of [32,128] ----
            x5t = tp.tile([P, CH, 43], F32, tag="x5t")
            for c in range(CH):
                pt = pstr.tile([P, 64], F32, tag="tr")
                nc.tensor.transpose(
                    pt[:, :64], x5cat[:, c * P : (c + 1) * P], ident[:64, :64]
                )
                if c % 2 == 0:
                    nc.vector.tensor_copy(x5t[:, c, :], pt[:, :43])
                else:
                    nc.scalar.copy(x5t[:, c, :], pt[:, :43])

            # ---- tail part 2: per-head analytic QP ----
            B10 = [P, CH, 10]
            alpha = x5t[:, :, 32:33]
            betas = x5t[:, :, 33:43]
            apb = t3("apb", 10)
            nc.vector.tensor_add(apb[:], betas, alpha.to_broadcast(B10))
            ab = t3("ab", 10)
            nc.vector.tensor_mul(ab[:], betas, alpha.to_broadcast(B10))
            hv = t3("hv", 10)
            nc.vector.tensor_mul(hv[:], apb[:], b_dot[:].to_broadcast(B10))
            hv2 = t3("hv2", 10)
            nc.vector.tensor_mul(hv2[:], ab[:], barrier[:].to_broadcast(B10))
            nc.vector.tensor_add(hv[:], hv[:], hv2[:])
            nc.vector.tensor_add(hv[:], hv[:], lf2b[:].to_broadcast(B10))

            refx = x5t[:, :, 0:20:2]
            refy = x5t[:, :, 1:20:2]
            viol = t3("viol", 10)
            v2t = t3("v2t", 10)
            nc.vector.tensor_mul(viol[:], refx, g1[:].to_broadcast(B10))
            nc.vector.tensor_mul(v2t[:], refy, g2[:].to_broadcast(B10))
            nc.vector.tensor_add(viol[:], viol[:], v2t[:])
            nc.vector.tensor_sub(viol[:], viol[:], hv[:])

            lam = t3("lam", 10)
            nc.vector.tensor_mul(lam[:], viol[:], igdot[:].to_broadcast(B10))
            nc.vector.tensor_scalar_max(lam[:], lam[:], 0.0)

            # S = sum_h wv_h * lam_h ; refbar = sum_h wv_h * ref_h
            wlam = t3("wlam", 10)
            nc.vector.tensor_mul(
                wlam[:], lam[:], wv20[:, None, 0::2].to_broadcast(B10)
            )
            S = t3("S")
            nc.vector.reduce_sum(S[:, :, 0], wlam[:], axis=AX.X)

            wref = t3("wref", 20)
            nc.vector.tensor_mul(
                wref[:], x5t[:, :, 0:20], wv20[:, None, :].to_broadcast([P, CH, 20])
            )
            rbx = t3("rbx")
            nc.vector.reduce_sum(rbx[:, :, 0], wref[:, :, 0::2], axis=AX.X)
            rby = t3("rby")
            nc.vector.reduce_sum(rby[:, :, 0], wref[:, :, 1::2], axis=AX.X)

            rtx = t3("rtx")
            nc.vector.tensor_mul(rtx[:], S[:], g1[:])
            nc.vector.tensor_sub(rtx[:], rtx[:], rbx[:])
            rty = t3("rty")
            nc.vector.tensor_mul(rty[:], S[:], g2[:])
            nc.vector.tensor_sub(rty[:], rty[:], rby[:])

            ot = t3("ot", 2)
            nc.vector.tensor_scalar(
                ot[:, :, 0], rtx[:, :, 0], mlt[:, 0:1], islt[:, 0:1],
                op0=ALU.subtract, op1=ALU.mult,
            )
            nc.vector.tensor_scalar(
                ot[:, :, 1], rty[:, :, 0], mlt[:, 1:2], islt[:, 1:2],
                op0=ALU.subtract, op1=ALU.mult,
            )
            nc.sync.dma_start(out, ot[:])

    nc.compile()
    return nc


def _get_nc():
    global _CACHED_NC
    if _CACHED_NC is None:
        _CACHED_NC = _build()
    return _CACHED_NC


def _bf16(a):
    return np.ascontiguousarray(a.astype(ml_dtypes.bfloat16))


def _f32(a):
    return np.ascontiguousarray(np.asarray(a, dtype=np.float32))


def _prep_inputs(inputs):
    x = _f32(inputs["x"])
    mean = _f32(inputs["mean"])
    std = _f32(inputs["std"])
    mean_label = _f32(inputs["mean_label"])
    std_label = _f32(inputs["std_label"])
    wt = _f32(inputs["wt"])
    W1, b1 = _f32(inputs["W1"]), _f32(inputs["b1"])
    W2, b2 = _f32(inputs["W2"]), _f32(inputs["b2"])
    W31, b31 = _f32(inputs["W31"]), _f32(inputs["b31"])
    W32, b32 = _f32(inputs["W32"]), _f32(inputs["b32"])
    W41, b41 = _f32(inputs["W41"]), _f32(inputs["b41"])
    W42, b42 = _f32(inputs["W42"]), _f32(inputs["b42"])
    W51, b51 = _f32(inputs["W51"]), _f32(inputs["b51"])
    W52, b52 = _f32(inputs["W52"]), _f32(inputs["b52"])

    def pack4(wT, KT, MT):  # (K, M) -> (128, MT, KT, 128)
        return _bf16(wT.reshape(KT, P, MT, P).transpose(1, 2, 0, 3))

    def pack3(wT, KT):  # (K, M) -> (128, KT, M)
        K, M = wT.shape
        return _bf16(wT.reshape(KT, P, M).transpose(1, 0, 2))

    W3T = np.concatenate([W31.T, W32.T], axis=1)  # (2048, 2048)
    b3 = np.concatenate([b31, b32])

    shared = {
        "w1": _bf16(W1.T),
        "w2": pack4(W2.T, 16, 16),
        "w3": pack4(W3T, 16, 16),
        "w41": pack4(W41.T, 8, 8),
        "w42": pack4(W42.T, 8, 8),
        "w51": pack3(W51.T, 8),
        "w52": pack3(W52.T, 8),
        "b1": _f32(b1.reshape(16, P).T),
        "b2": _f32(b2.reshape(16, P).T),
        "b3": _f32(b3.reshape(16, P).T),
        "b41": _f32(b41.reshape(8, P).T),
        "b42": _f32(b42.reshape(8, P).T),
        "b51": b51,
        "b52": b52,
        "stdb": _f32(np.tile(std[None, :], (P, 1))),
        "meanb": _f32(np.tile(mean[None, :], (P, 1))),
        "mlb": _f32(np.tile(mean_label[None, :], (P, 1))),
        "islb": _f32(np.tile((1.0 / std_label)[None, :], (P, 1))),
        "wtv": wt,
    }

    in_maps = []
    for i in range(N_CORES):
        xs = x[i * B : (i + 1) * B]  # (2048, 4)
        m = dict(shared)
        m["xt"] = _bf16(xs.T)
        m["xn"] = _f32(xs.reshape(CH, P, 4).transpose(1, 0, 2))
        in_maps.append(m)
    return in_maps


def kernel_core(inputs, trace=False):
    nc = _get_nc()
    in_maps = _prep_inputs(inputs)
    res = run_bass_kernel_spmd(
        nc, in_maps, core_ids=list(range(N_CORES)), trace=trace
    )
    shards = []
    for i in range(N_CORES):
        o = res.results[i]["out"]  # (128, 16, 2)
        shards.append(o.transpose(1, 0, 2).reshape(B, 2))
    return np.concatenate(shards, axis=0).astype(np.float32), res


def kernel(**inputs):
    out, _ = kernel_core(inputs)
    return out


# revision 17
# speedup vs baseline: 1.0084x; 1.0084x over previous
"""Trainium2 Bass kernel for nn_ABNet_U (multi-branch MLP + CBF-QP head).

Data-parallel over batch: 16384 rows -> 8 NeuronCores x 2048 rows.
Weights replicated, host-prepped into K-major bf16 layouts; all GEMMs run
on the TensorEngine with fp32 PSUM accumulation and fused bias+activation
eviction on the Scalar/Vector engines.  The trig/QP tail runs in fp32 on
the Vector/Scalar engines with batch on partitions, emitted early so it
overlaps the GEMM phase.
"""

import sys

sys.path.insert(0, "/opt/trn_rl_repo")

import numpy as np
import ml_dtypes

import concourse.bass as bass
import concourse.mybir as mybir
import concourse.tile as tile
from concourse import bacc
from concourse.bass_utils import run_bass_kernel_spmd
from concourse.masks import make_identity

N_CORES = 8
B_GLOBAL = 16384
B = B_GLOBAL // N_CORES  # 2048 rows per core
P = 128
CH = B // P              # 16 batch chunks of 128 (tail layout)
NF = 512                 # matmul free-dim chunk
NB = B // NF             # 4 free chunks
HEADS = 10

AF = mybir.ActivationFunctionType
ALU = mybir.AluOpType
AX = mybir.AxisListType
F32 = mybir.dt.float32
BF16 = mybir.dt.bfloat16
I32 = mybir.dt.int32

TWO_PI = float(2.0 * np.pi)
HALF_PI = float(0.5 * np.pi)

_CACHED_NC = None


def _build():
    nc = bacc.Bacc(
        "TRN2",
        target_bir_lowering=False,
        debug=False,
        enable_asserts=False,
        num_devices=N_CORES,
    )

    def din(name, shape, dt=F32):
        return nc.dram_tensor(name, list(shape), dt, kind="ExternalInput").ap()

    xt = din("xt", (P, B), BF16)
    xn = din("xn", (P, CH, 4))              # x shard, [p, chunk, feat] fp32
    w1 = din("w1", (P, 2048), BF16)
    w2 = din("w2", (P, 16, 16, P), BF16)    # [p, mt, kt, mc]
    w3 = din("w3", (P, 16, 16, P), BF16)
    w41 = din("w41", (P, 8, 8, P), BF16)
    w42 = din("w42", (P, 8, 8, P), BF16)
    w51 = din("w51", (P, 8, 20), BF16)      # [p, kt, m]
    w52 = din("w52", (P, 8, 11), BF16)
    b1 = din("b1", (P, 16))
    b2 = din("b2", (P, 16))
    b3 = din("b3", (P, 16))
    b41 = din("b41", (P, 8))
    b42 = din("b42", (P, 8))
    b51 = din("b51", (20,))
    b52 = din("b52", (11,))
    stdb = din("stdb", (P, 4))
    meanb = din("meanb", (P, 4))
    mlb = din("mlb", (P, 2))
    islb = din("islb", (P, 2))
    wtv = din("wtv", (10,))
    out = nc.dram_tensor("out", [P, CH, 2], F32, kind="ExternalOutput").ap()

    with tile.TileContext(nc) as tc:
        from contextlib import ExitStack

        with ExitStack() as ctx:
            const = ctx.enter_context(tc.tile_pool(name="const", bufs=1))
            wpool = ctx.enter_context(tc.tile_pool(name="wpool", bufs=3))
            hpool = ctx.enter_context(tc.tile_pool(name="hpool", bufs=2))
            psum = ctx.enter_context(tc.tile_pool(name="psum", bufs=4, space="PSUM"))
            tp = ctx.enter_context(tc.tile_pool(name="tp", bufs=1))

            # ---- L1-critical loads first: keep the PE fed from t=0 ----
            b1t = const.tile([P, 16], F32, tag="b1")
            nc.sync.dma_start(b1t[:], b1)
            xtb = const.tile([P, B], BF16, tag="xtb")
            nc.sync.dma_start(xtb[:], xt)
            w1tb = const.tile([P, 2048], BF16, tag="w1tb")
            nc.sync.dma_start(w1tb[:], w1)

            # prefetch: L2's first weight column gates the L1->L2 handoff;
            # the tiny L5 weight tiles ride the otherwise-idle gpsimd queue.
            wcol2_0 = wpool.tile([P, 16, P], BF16, tag="wcol", name="wcol2_0")
            nc.sync.dma_start(wcol2_0[:], w2[:, 0])
            w51c = const.tile([P, 8, 20], BF16, tag="w51c")
            nc.gpsimd.dma_start(w51c[:], w51)
            w52c = const.tile([P, 8, 11], BF16, tag="w52c")
            nc.gpsimd.dma_start(w52c[:], w52)

            # PE warm-up: dummy matmuls fill the idle window while the L1
            # input DMAs land, and push the HAM clock gate to 8/8 before the
            # real matmul stream begins.
            wrm = const.tile([P, NF], BF16, tag="wrm")
            nc.vector.memset(wrm[:], 0.0)
            wps = psum.tile([P, 2 * NF], F32, tag="mm", name="wps")
            for _ in range(20):
                nc.tensor.matmul(
                    wps[:, :NF], wrm[:, :P], wrm[:], start=True, stop=True
                )

            # warm the ACT engine's function tables in the startup window so
            # the 1.3us table loads don't land mid-stream later
            tbl = const.tile([1, 2], F32, tag="tbl")
            nc.vector.memset(tbl[:], 0.25)
            nc.scalar.activation(tbl[:, 1:2], tbl[:, 0:1], AF.Relu)
            nc.scalar.activation(tbl[:, 1:2], tbl[:, 0:1], AF.Exp)
            nc.scalar.activation(tbl[:, 1:2], tbl[:, 0:1], AF.Sin)
            nc.scalar.activation(tbl[:, 1:2], tbl[:, 0:1], AF.Sigmoid)
            nc.scalar.activation(tbl[:, 1:2], tbl[:, 0:1], AF.Identity)

            # ---- L1: h1 = relu(W1 @ x^T + b1), K=4 zero-padded to 128 ----
            # One matmul per eviction, so evictions bound this layer: split
            # them across the Scalar and Vector engines.
            h1 = hpool.tile([P, 16, B], BF16, tag="act")
            flip = 0
            for g in range(NB // 2):
                for m in range(16):
                    ps = psum.tile([P, 2 * NF], F32, tag="mm")
                    for half in range(2):
                        n = 2 * g + half
                        nc.tensor.matmul(
                            ps[:, half * NF : (half + 1) * NF],
                            w1tb[:, m * P : (m + 1) * P],
                            xtb[:, n * NF : (n + 1) * NF],
                            start=True,
                            stop=True,
                        )
                    dst = h1[:, m, 2 * g * NF : 2 * (g + 1) * NF]
                    if flip % 2 == 0:
                        nc.scalar.activation(
                            dst, ps[:], AF.Relu, bias=b1t[:, m : m + 1]
                        )
                    else:
                        nc.vector.tensor_scalar(
                            dst, ps[:], b1t[:, m : m + 1], 0.0,
                            op0=ALU.add, op1=ALU.max,
                        )
                    flip += 1

            # ---- remaining constants (emitted after L1 so they never gate it)
            b2t = const.tile([P, 16], F32, tag="b2")
            nc.sync.dma_start(b2t[:], b2)
            b3t = const.tile([P, 16], F32, tag="b3")
            nc.sync.dma_start(b3t[:], b3)
            b41t = const.tile([P, 8], F32, tag="b41")
            nc.sync.dma_start(b41t[:], b41)
            b42t = const.tile([P, 8], F32, tag="b42")
            nc.sync.dma_start(b42t[:], b42)
            b51t = const.tile([20, 1], F32, tag="b51")
            nc.sync.dma_start(b51t[:], b51[:, None])
            b52t = const.tile([11, 1], F32, tag="b52")
            nc.sync.dma_start(b52t[:], b52[:, None])
            stdt = const.tile([P, 4], F32, tag="stdt")
            nc.sync.dma_start(stdt[:], stdb)
            meant = const.tile([P, 4], F32, tag="meant")
            nc.sync.dma_start(meant[:], meanb)
            mlt = const.tile([P, 2], F32, tag="mlt")
            nc.sync.dma_start(mlt[:], mlb)
            islt = const.tile([P, 2], F32, tag="islt")
            nc.sync.dma_start(islt[:], islb)
            halfpi = const.tile([P, 1], F32, tag="halfpi")
            nc.vector.memset(halfpi[:], HALF_PI)
            ident = const.tile([P, P], F32)
            make_identity(nc, ident[:])

            # softmax(wt) DVE chain (PE broadcast deferred until after L4)
            wtt = const.tile([1, 10], F32, tag="wtt")
            nc.sync.dma_start(wtt[:], wtv[None, :])
            mx = const.tile([1, 1], F32, tag="mx")
            nc.vector.reduce_max(mx[:, 0:1], wtt[:], axis=AX.X)
            nm = const.tile([1, 1], F32, tag="nm")
            nc.vector.tensor_scalar_mul(nm[:], mx[:], -1.0)
            ex = const.tile([1, 10], F32, tag="ex")
            nc.scalar.activation(ex[:], wtt[:], AF.Exp, bias=nm[:])
            sm = const.tile([1, 1], F32, tag="sm")
            nc.vector.reduce_sum(sm[:, 0:1], ex[:], axis=AX.X)
            inv = const.tile([1, 1], F32, tag="inv")
            nc.vector.reciprocal(inv[:], sm[:])
            wv10 = const.tile([1, 10], F32, tag="wv10")
            nc.vector.tensor_scalar_mul(wv10[:], ex[:], inv[:])
            wvp = const.tile([32, 32], F32, tag="wvp")
            nc.vector.memset(wvp[:], 0.0)
            nc.vector.tensor_copy(
                wvp[0:1, 0:20].rearrange("p (h c) -> p h c", c=2),
                wv10[:, :, None].to_broadcast([1, 10, 2]),
            )
            onesp = const.tile([32, P], F32, tag="onesp")
            nc.vector.memset(onesp[:], 0.0)
            nc.vector.memset(onesp[0:1, :], 1.0)

            # combined x51/x52 head tile (rows 0..19 = x51, 20..30 = x52)
            x5cat = tp.tile([64, B], F32, tag="x5cat")
            nc.vector.memset(x5cat[:], 0.0)

            # ---- tail part 1: geometry from x only — emitted now so the
            # Vector engine computes it underneath the L2..L5 GEMMs.
            def t3(tag, d=1):
                return tp.tile([P, CH, d], F32, tag=tag, name=tag)

            xnt = t3("xnt", 4)
            nc.sync.dma_start(xnt[:], xn)
            x0 = t3("x0", 4)
            nc.vector.tensor_mul(
                x0[:], xnt[:], stdt[:, None, :].to_broadcast([P, CH, 4])
            )
            nc.vector.tensor_add(
                x0[:], x0[:], meant[:, None, :].to_broadcast([P, CH, 4])
            )

            th = x0[:, :, 0::2]   # [P, CH, 2] angles
            wv_ = x0[:, :, 1::2]  # [P, CH, 2] angular velocities

            # range-reduce th -> rs in [-pi, pi]:  rs = th - 2pi*rint(th/2pi)
            q = t3("q", 2)
            qi = tp.tile([P, CH, 2], I32, tag="qi")
            qr = t3("qr", 2)
            rs = t3("rs", 2)
            nc.vector.tensor_scalar_mul(q[:], th, 1.0 / TWO_PI)
            nc.vector.tensor_copy(qi[:], q[:])
            nc.vector.tensor_copy(qr[:], qi[:])
            nc.vector.scalar_tensor_tensor(
                rs[:], in0=qr[:], scalar=-TWO_PI, in1=th,
                op0=ALU.mult, op1=ALU.add,
            )
            # range-reduce th + pi/2 -> rc (for cos)
            qc = t3("qc", 2)
            qci = tp.tile([P, CH, 2], I32, tag="qci")
            qcr = t3("qcr", 2)
            rc = t3("rc", 2)
            nc.vector.tensor_scalar(
                qc[:], th, 1.0 / TWO_PI, 0.25, op0=ALU.mult, op1=ALU.add
            )
            nc.vector.tensor_copy(qci[:], qc[:])
            nc.vector.tensor_copy(qcr[:], qci[:])
            nc.vector.scalar_tensor_tensor(
                rc[:], in0=qcr[:], scalar=-TWO_PI, in1=th,
                op0=ALU.mult, op1=ALU.add,
            )
            nc.vector.tensor_scalar_add(rc[:], rc[:], HALF_PI)

            sn = t3("sn", 2)
            cs = t3("cs", 2)
            nc.scalar.activation(sn[:], rs[:], AF.Sin)
            nc.scalar.activation(cs[:], rc[:], AF.Sin)

            s1, s2 = sn[:, :, 0:1], sn[:, :, 1:2]
            c1, c2 = cs[:, :, 0:1], cs[:, :, 1:2]
            w1v, w2v = wv_[:, :, 0:1], wv_[:, :, 1:2]

            px = t3("px")
            nc.vector.tensor_add(px[:], c1, c2)
            nc.vector.tensor_scalar_mul(px[:], px[:], 3.0)
            py = t3("py")
            nc.vector.tensor_add(py[:], s1, s2)
            nc.vector.tensor_scalar(py[:], py[:], 3.0, -7.0, op0=ALU.mult, op1=ALU.add)

            s1w = t3("s1w")
            nc.vector.tensor_mul(s1w[:], s1, w1v)
            s2w = t3("s2w")
            nc.vector.tensor_mul(s2w[:], s2, w2v)
            vx = t3("vx")
            nc.vector.tensor_add(vx[:], s1w[:], s2w[:])
            nc.vector.tensor_scalar_mul(vx[:], vx[:], -3.0)
            c1w = t3("c1w")
            nc.vector.tensor_mul(c1w[:], c1, w1v)
            c2w = t3("c2w")
            nc.vector.tensor_mul(c2w[:], c2, w2v)
            vy = t3("vy")
            nc.vector.tensor_add(vy[:], c1w[:], c2w[:])
            nc.vector.tensor_scalar_mul(vy[:], vy[:], 3.0)

            pxx = t3("pxx")
            nc.vector.tensor_mul(pxx[:], px[:], px[:])
            pyy = t3("pyy")
            nc.vector.tensor_mul(pyy[:], py[:], py[:])
            # barrier scaled by 16 = alpha*beta scale (4*sigmoid each)
            barrier = t3("barrier")
            nc.vector.tensor_add(barrier[:], pxx[:], pyy[:])
            nc.vector.tensor_scalar(
                barrier[:], barrier[:], 16.0, -256.0, op0=ALU.mult, op1=ALU.add
            )

            pv1 = t3("pv1")
            nc.vector.tensor_mul(pv1[:], px[:], vx[:])
            pv2 = t3("pv2")
            nc.vector.tensor_mul(pv2[:], py[:], vy[:])
            b_dot = t3("b_dot")
            nc.vector.tensor_add(b_dot[:], pv1[:], pv2[:])
            # 2 (from b_dot) * 4 (alpha+beta sigmoid scale)
            nc.vector.tensor_scalar_mul(b_dot[:], b_dot[:], 8.0)

            w1sq = t3("w1sq")
            nc.vector.tensor_mul(w1sq[:], w1v, w1v)
            w2sq = t3("w2sq")
            nc.vector.tensor_mul(w2sq[:], w2v, w2v)
            ca = t3("ca")
            nc.vector.tensor_mul(ca[:], c1, w1sq[:])
            cb = t3("cb")
            nc.vector.tensor_mul(cb[:], c2, w2sq[:])
            nc.vector.tensor_add(ca[:], ca[:], cb[:])   # c1*w1^2 + c2*w2^2
            sa = t3("sa")
            nc.vector.tensor_mul(sa[:], s1, w1sq[:])
            sb = t3("sb")
            nc.vector.tensor_mul(sb[:], s2, w2sq[:])
            nc.vector.tensor_add(sa[:], sa[:], sb[:])   # s1*w1^2 + s2*w2^2

            vxx = t3("vxx")
            nc.vector.tensor_mul(vxx[:], vx[:], vx[:])
            vyy = t3("vyy")
            nc.vector.tensor_mul(vyy[:], vy[:], vy[:])
            vsum = t3("vsum")
            nc.vector.tensor_add(vsum[:], vxx[:], vyy[:])
            nc.vector.tensor_scalar_mul(vsum[:], vsum[:], 2.0)  # 2vx^2+2vy^2

            pca = t3("pca")
            nc.vector.tensor_mul(pca[:], px[:], ca[:])
            psa = t3("psa")
            nc.vector.tensor_mul(psa[:], py[:], sa[:])
            nc.vector.tensor_add(pca[:], pca[:], psa[:])
            lf2b = t3("lf2b")
            nc.vector.scalar_tensor_tensor(
                lf2b[:], in0=pca[:], scalar=-6.0, in1=vsum[:],
                op0=ALU.mult, op1=ALU.add,
            )  # Lf2b = 2(vx^2+vy^2) - 6*(px*ca + py*sa)

            g1 = t3("g1")
            m1 = t3("m1")
            nc.vector.tensor_mul(m1[:], py[:], c1)
            m2 = t3("m2")
            nc.vector.tensor_mul(m2[:], px[:], s1)
            nc.vector.tensor_sub(g1[:], m1[:], m2[:])
            nc.vector.tensor_scalar_mul(g1[:], g1[:], 6.0)
            g2 = t3("g2")
            nc.vector.tensor_mul(m1[:], py[:], c2)
            nc.vector.tensor_mul(m2[:], px[:], s2)
            nc.vector.tensor_sub(g2[:], m1[:], m2[:])
            nc.vector.tensor_scalar_mul(g2[:], g2[:], 6.0)

            gdot = t3("gdot")
            g1sq = t3("g1sq")
            nc.vector.tensor_mul(g1sq[:], g1[:], g1[:])
            g2sq = t3("g2sq")
            nc.vector.tensor_mul(g2sq[:], g2[:], g2[:])
            nc.vector.tensor_add(gdot[:], g1sq[:], g2sq[:])
            igdot = t3("igdot")
            nc.vector.reciprocal(igdot[:], gdot[:])

            # ---- generic streamed GEMM layer ----
            def mlp_layer(wdram, KT, MT, MD, hin, kin_base, btile, evict,
                          prefetched=None):
                for m in range(MT):
                    mp = min(P, MD - m * P)
                    if prefetched is not None and m in prefetched:
                        wcol = prefetched[m]
                    else:
                        wcol = wpool.tile([P, KT, mp], BF16, tag="wcol")
                        if len(wdram.shape) == 4:
                            nc.sync.dma_start(wcol[:], wdram[:, m])
                        else:
                            nc.sync.dma_start(wcol[:], wdram)
                    for g in range(NB // 2):
                        ps = psum.tile([P, 2 * NF], F32, tag="mm")
                        for half in range(2):
                            n = 2 * g + half
                            for k in range(KT):
                                nc.tensor.matmul(
                                    ps[:mp, half * NF : (half + 1) * NF],
                                    wcol[:, k, :],
                                    hin[:, kin_base + k, n * NF : (n + 1) * NF],
                                    start=(k == 0),
                                    stop=(k == KT - 1),
                                )
                        evict(m, g, ps[:mp])

            # ---- L2 / L3 / L4 ----
            h2 = hpool.tile([P, 16, B], BF16, tag="act")

            def ev_h(hout, btile, m_off=0):
                def _e(m, g, ps):
                    nc.scalar.activation(
                        hout[:, m_off + m, 2 * g * NF : 2 * (g + 1) * NF], ps,
                        AF.Relu, bias=btile[:, m : m + 1],
                    )
                return _e

            mlp_layer(w2, 16, 16, 2048, h1, 0, b2t, ev_h(h2, b2t), prefetched={0: wcol2_0})

            h3 = hpool.tile([P, 16, B], BF16, tag="act")
            mlp_layer(w3, 16, 16, 2048, h2, 0, b3t, ev_h(h3, b3t))

            h4 = hpool.tile([P, 16, B], BF16, tag="act")
            mlp_layer(w41, 8, 8, 1024, h3, 0, b41t, ev_h(h4, b41t, 0))
            mlp_layer(w42, 8, 8, 1024, h3, 8, b42t, ev_h(h4, b42t, 8))

            # ---- L5 into the combined head tile ----
            def ev_51(m, g, ps):
                nc.vector.tensor_scalar_add(
                    x5cat[:20, 2 * g * NF : 2 * (g + 1) * NF], ps, b51t[:]
                )

            def ev_52(m, g, ps):
                nc.scalar.activation(
                    x5cat[32:43, 2 * g * NF : 2 * (g + 1) * NF], ps, AF.Sigmoid,
                    bias=b52t[:],
                )

            mlp_layer(w51, 8, 1, 20, h4, 0, b51t, ev_51, prefetched={0: w51c})
            mlp_layer(w52, 8, 1, 11, h4, 8, b52t, ev_52, prefetched={0: w52c})

            # wv broadcast to all partitions (PE hits this after L4/L5)
            pwv = psum.tile([P, 2 * NF], F32, tag="mm", name="pwv")
            nc.tensor.matmul(pwv[:, :32], onesp[:], wvp[:], start=True, stop=True)
            wv20 = const.tile([P, 20], F32, tag="wv20")
            nc.vector.tensor_copy(wv20[:], pwv[:, :20])

            # ---- transpose the combined head tile: 16 chunks of [32,128] ----
            x5t = tp.tile([P, CH, 43], F32, tag="x5t")
            for c in range(CH):
                pt = psum.tile([P, 2 * NF], F32, tag="mm", name="pt")
                nc.tensor.transpose(
                    pt[:, :64], x5cat[:, c * P : (c + 1) * P], ident[:64, :64]
                )
                if c % 2 == 0:
                    nc.vector.tensor_copy(x5t[:, c, :], pt[:, :43])
                else:
                    nc.scalar.copy(x5t[:, c, :], pt[:, :43])

            # ---- tail part 2: per-head analytic QP ----
            B10 = [P, CH, 10]
            alpha = x5t[:, :, 32:33]
            betas = x5t[:, :, 33:43]
            apb = t3("apb", 10)
            nc.vector.tensor_add(apb[:], betas, alpha.to_broadcast(B10))
            ab = t3("ab", 10)
            nc.vector.tensor_mul(ab[:], betas, alpha.to_broadcast(B10))
            hv = t3("hv", 10)
            nc.vector.tensor_mul(hv[:], apb[:], b_dot[:].to_broadcast(B10))
            hv2 = t3("hv2", 10)
            nc.vector.tensor_mul(hv2[:], ab[:], barrier[:].to_broadcast(B10))
            nc.vector.tensor_add(hv[:], hv[:], hv2[:])
            nc.vector.tensor_add(hv[:], hv[:], lf2b[:].to_broadcast(B10))

            refx = x5t[:, :, 0:20:2]
            refy = x5t[:, :, 1:20:2]
            viol = t3("viol", 10)
            v2t = t3("v2t", 10)
            nc.vector.tensor_mul(viol[:], refx, g1[:].to_broadcast(B10))
            nc.vector.tensor_mul(v2t[:], refy, g2[:].to_broadcast(B10))
            nc.vector.tensor_add(viol[:], viol[:], v2t[:])
            nc.vector.tensor_sub(viol[:], viol[:], hv[:])

            lam = t3("lam", 10)
            nc.vector.tensor_mul(lam[:], viol[:], igdot[:].to_broadcast(B10))
            nc.vector.tensor_scalar_max(lam[:], lam[:], 0.0)

            # S = sum_h wv_h * lam_h ; refbar = sum_h wv_h * ref_h
            wlam = t3("wlam", 10)
            nc.vector.tensor_mul(
                wlam[:], lam[:], wv20[:, None, 0::2].to_broadcast(B10)
            )
            S = t3("S")
            nc.vector.reduce_sum(S[:, :, 0], wlam[:], axis=AX.X)

            wref = t3("wref", 20)
            nc.vector.tensor_mul(
                wref[:], x5t[:, :, 0:20], wv20[:, None, :].to_broadcast([P, CH, 20])
            )
            rbx = t3("rbx")
            nc.vector.reduce_sum(rbx[:, :, 0], wref[:, :, 0::2], axis=AX.X)
            rby = t3("rby")
            nc.vector.reduce_sum(rby[:, :, 0], wref[:, :, 1::2], axis=AX.X)

            rtx = t3("rtx")
            nc.vector.tensor_mul(rtx[:], S[:], g1[:])
            nc.vector.tensor_sub(rtx[:], rtx[:], rbx[:])
            rty = t3("rty")
            nc.vector.tensor_mul(rty[:], S[:], g2[:])
            nc.vector.tensor_sub(rty[:], rty[:], rby[:])

            ot = t3("ot", 2)
            nc.vector.tensor_scalar(
                ot[:, :, 0], rtx[:, :, 0], mlt[:, 0:1], islt[:, 0:1],
                op0=ALU.subtract, op1=ALU.mult,
            )
            nc.vector.tensor_scalar(
                ot[:, :, 1], rty[:, :, 0], mlt[:, 1:2], islt[:, 1:2],
                op0=ALU.subtract, op1=ALU.mult,
            )
            nc.sync.dma_start(out, ot[:])

    nc.compile()
    return nc


def _get_nc():
    global _CACHED_NC
    if _CACHED_NC is None:
        _CACHED_NC = _build()
    return _CACHED_NC


def _bf16(a):
    return np.ascontiguousarray(a.astype(ml_dtypes.bfloat16))


def _f32(a):
    return np.ascontiguousarray(np.asarray(a, dtype=np.float32))


def _prep_inputs(inputs):
    x = _f32(inputs["x"])
    mean = _f32(inputs["mean"])
    std = _f32(inputs["std"])
    mean_label = _f32(inputs["mean_label"])
    std_label = _f32(inputs["std_label"])
    wt = _f32(inputs["wt"])
    W1, b1 = _f32(inputs["W1"]), _f32(inputs["b1"])
    W2, b2 = _f32(inputs["W2"]), _f32(inputs["b2"])
    W31, b31 = _f32(inputs["W31"]), _f32(inputs["b31"])
    W32, b32 = _f32(inputs["W32"]), _f32(inputs["b32"])
    W41, b41 = _f32(inputs["W41"]), _f32(inputs["b41"])
    W42, b42 = _f32(inputs["W42"]), _f32(inputs["b42"])
    W51, b51 = _f32(inputs["W51"]), _f32(inputs["b51"])
    W52, b52 = _f32(inputs["W52"]), _f32(inputs["b52"])

    def pack4(wT, KT, MT):  # (K, M) -> (128, MT, KT, 128)
        return _bf16(wT.reshape(KT, P, MT, P).transpose(1, 2, 0, 3))

    def pack3(wT, KT):  # (K, M) -> (128, KT, M)
        K, M = wT.shape
        return _bf16(wT.reshape(KT, P, M).transpose(1, 0, 2))

    W3T = np.concatenate([W31.T, W32.T], axis=1)  # (2048, 2048)
    b3 = np.concatenate([b31, b32])

    w1p = np.zeros((P, 2048), np.float32)
    w1p[:4] = W1.T
    shared = {
        "w1": _bf16(w1p),
        "w2": pack4(W2.T, 16, 16),
        "w3": pack4(W3T, 16, 16),
        "w41": pack4(W41.T, 8, 8),
        "w42": pack4(W42.T, 8, 8),
        "w51": pack3(W51.T, 8),
        "w52": pack3(W52.T, 8),
        "b1": _f32(b1.reshape(16, P).T),
        "b2": _f32(b2.reshape(16, P).T),
        "b3": _f32(b3.reshape(16, P).T),
        "b41": _f32(b41.reshape(8, P).T),
        "b42": _f32(b42.reshape(8, P).T),
        "b51": b51,
        "b52": b52,
        "stdb": _f32(np.tile(std[None, :], (P, 1))),
        "meanb": _f32(np.tile(mean[None, :], (P, 1))),
        "mlb": _f32(np.tile(mean_label[None, :], (P, 1))),
        "islb": _f32(np.tile((1.0 / std_label)[None, :], (P, 1))),
        "wtv": wt,
    }

    in_maps = []
    for i in range(N_CORES):
        xs = x[i * B : (i + 1) * B]  # (2048, 4)
        m = dict(shared)
        xtp = np.zeros((P, B), np.float32)
        xtp[:4] = xs.T
        m["xt"] = _bf16(xtp)
        m["xn"] = _f32(xs.reshape(CH, P, 4).transpose(1, 0, 2))
        in_maps.append(m)
    return in_maps


def kernel_core(inputs, trace=False):
    nc = _get_nc()
    in_maps = _prep_inputs(inputs)
    res = run_bass_kernel_spmd(
        nc, in_maps, core_ids=list(range(N_CORES)), trace=trace
    )
    shards = []
    for i in range(N_CORES):
        o = res.results[i]["out"]  # (128, 16, 2)
        shards.append(o.transpose(1, 0, 2).reshape(B, 2))
    return np.concatenate(shards, axis=0).astype(np.float32), res


def kernel(**inputs):
    out, _ = kernel_core(inputs)
    return out


# revision 21
# speedup vs baseline: 1.8326x; 1.8173x over previous
"""Trainium2 Bass kernel for nn_ABNet_U (multi-branch MLP + CBF-QP head).

Data-parallel over batch: 16384 rows -> 8 NeuronCores x 2048 rows.
Weights replicated, host-prepped into K-major bf16 layouts; all GEMMs run
on the TensorEngine with fp32 PSUM accumulation and fused bias+activation
eviction on the Scalar/Vector engines.  The trig/QP tail runs in fp32 on
the Vector/Scalar engines with batch on partitions, emitted early so it
overlaps the GEMM phase.
"""

import sys

sys.path.insert(0, "/opt/trn_rl_repo")

import numpy as np
import ml_dtypes

import concourse.bass as bass
import concourse.mybir as mybir
import concourse.tile as tile
from concourse import bacc
from concourse.bass_utils import run_bass_kernel_spmd
from concourse.masks import make_identity

N_CORES = 8
B_GLOBAL = 16384
B = B_GLOBAL // N_CORES  # 2048 rows per core
P = 128
CH = B // P              # 16 batch chunks of 128 (tail layout)
NF = 512                 # matmul free-dim chunk
NB = B // NF             # 4 free chunks
HEADS = 10

AF = mybir.ActivationFunctionType
ALU = mybir.AluOpType
AX = mybir.AxisListType
F32 = mybir.dt.float32
BF16 = mybir.dt.bfloat16
I32 = mybir.dt.int32

TWO_PI = float(2.0 * np.pi)
HALF_PI = float(0.5 * np.pi)

# Layers computed in fp8-e4m3 with DoubleRow (2 weights/PE cell, halves the
# matmul count).  Weights use per-tensor scales; activations use static
# per-layer scales calibrated host-side on a row subsample with 4.7x
# headroom to the e4m3 max.
FP8_LAYERS = frozenset({"l2", "l3", "l4"})  # override via configure()
FP8 = mybir.dt.float8e4

_CACHED_NC = None


def _build(fp8_layers=frozenset()):
    nc = bacc.Bacc(
        "TRN2",
        target_bir_lowering=False,
        debug=False,
        enable_asserts=False,
        num_devices=N_CORES,
    )

    def din(name, shape, dt=F32):
        return nc.dram_tensor(name, list(shape), dt, kind="ExternalInput").ap()

    xt = din("xt", (P, B), BF16)
    xn = din("xn", (P, CH, 4))              # x shard, [p, chunk, feat] fp32
    w1 = din("w1", (P, 2048), BF16)
    w2 = din("w2", (P, 16, 16, P), FP8 if "l2" in fp8_layers else BF16)
    w3 = din("w3", (P, 16, 16, P), FP8 if "l3" in fp8_layers else BF16)
    w41 = din("w41", (P, 8, 8, P), FP8 if "l4" in fp8_layers else BF16)
    w42 = din("w42", (P, 8, 8, P), FP8 if "l4" in fp8_layers else BF16)
    esc2 = din("esc2", (P, 1))
    esc3 = din("esc3", (P, 1))
    esc4 = din("esc4", (P, 1))
    w51 = din("w51", (P, 8, 20), BF16)      # [p, kt, m]
    w52 = din("w52", (P, 8, 11), BF16)
    b1 = din("b1", (P, 16))
    b2 = din("b2", (P, 16))
    b3 = din("b3", (P, 16))
    b41 = din("b41", (P, 8))
    b42 = din("b42", (P, 8))
    b51 = din("b51", (20,))
    b52 = din("b52", (11,))
    stdb = din("stdb", (P, 4))
    meanb = din("meanb", (P, 4))
    mlb = din("mlb", (P, 2))
    islb = din("islb", (P, 2))
    wtv = din("wtv", (10,))
    out = nc.dram_tensor("out", [P, CH, 2], F32, kind="ExternalOutput").ap()

    with tile.TileContext(nc) as tc:
        from contextlib import ExitStack

        with ExitStack() as ctx:
            const = ctx.enter_context(tc.tile_pool(name="const", bufs=1))
            wpool = ctx.enter_context(tc.tile_pool(name="wpool", bufs=3))
            hpool = ctx.enter_context(tc.tile_pool(name="hpool", bufs=2))
            psum = ctx.enter_context(tc.tile_pool(name="psum", bufs=4, space="PSUM"))
            tp = ctx.enter_context(tc.tile_pool(name="tp", bufs=1))

            # ---- L1-critical loads first: keep the PE fed from t=0 ----
            b1t = const.tile([P, 16], F32, tag="b1")
            nc.sync.dma_start(b1t[:], b1)
            xtb = const.tile([P, B], BF16, tag="xtb")
            nc.sync.dma_start(xtb[:], xt)
            w1tb = const.tile([P, 2048], BF16, tag="w1tb")
            nc.sync.dma_start(w1tb[:], w1)

            # prefetch: L2's first weight column gates the L1->L2 handoff;
            # the tiny L5 weight tiles ride the otherwise-idle gpsimd queue.
            wcol2_0 = wpool.tile(
                [P, 16, P], FP8 if "l2" in fp8_layers else BF16,
                tag="wcol", name="wcol2_0",
            )
            nc.sync.dma_start(wcol2_0[:], w2[:, 0])
            w51c = const.tile([P, 8, 20], BF16, tag="w51c")
            nc.gpsimd.dma_start(w51c[:], w51)
            w52c = const.tile([P, 8, 11], BF16, tag="w52c")
            nc.gpsimd.dma_start(w52c[:], w52)

            # PE warm-up: dummy matmuls fill the idle window while the L1
            # input DMAs land, and push the HAM clock gate to 8/8 before the
            # real matmul stream begins.
            wrm = const.tile([P, NF], BF16, tag="wrm")
            nc.vector.memset(wrm[:], 0.0)
            wps = psum.tile([P, 2 * NF], F32, tag="mm", name="wps")
            for _ in range(20):
                nc.tensor.matmul(
                    wps[:, :NF], wrm[:, :P], wrm[:], start=True, stop=True
                )

            # warm the ACT engine's function tables in the startup window so
            # the 1.3us table loads don't land mid-stream later
            tbl = const.tile([1, 2], F32, tag="tbl")
            nc.vector.memset(tbl[:], 0.25)
            nc.scalar.activation(tbl[:, 1:2], tbl[:, 0:1], AF.Relu)
            nc.scalar.activation(tbl[:, 1:2], tbl[:, 0:1], AF.Exp)
            nc.scalar.activation(tbl[:, 1:2], tbl[:, 0:1], AF.Sin)
            nc.scalar.activation(tbl[:, 1:2], tbl[:, 0:1], AF.Sigmoid)
            nc.scalar.activation(tbl[:, 1:2], tbl[:, 0:1], AF.Identity)

            # ---- L1: h1 = relu(W1 @ x^T + b1), K=4 zero-padded to 128 ----
            # One matmul per eviction, so evictions bound this layer: split
            # them across the Scalar and Vector engines.
            h1dt = FP8 if "l2" in fp8_layers else BF16
            h2dt = FP8 if "l3" in fp8_layers else BF16
            h3dt = FP8 if "l4" in fp8_layers else BF16
            h1 = hpool.tile([P, 16, B], h1dt, tag="act", name="h1")
            flip = 0
            for g in range(NB // 2):
                for m in range(16):
                    ps = psum.tile([P, 2 * NF], F32, tag="mm")
                    for half in range(2):
                        n = 2 * g + half
                        nc.tensor.matmul(
                            ps[:, half * NF : (half + 1) * NF],
                            w1tb[:, m * P : (m + 1) * P],
                            xtb[:, n * NF : (n + 1) * NF],
                            start=True,
                            stop=True,
                        )
                    dst = h1[:, m, 2 * g * NF : 2 * (g + 1) * NF]
                    if flip % 2 == 0:
                        nc.scalar.activation(
                            dst, ps[:], AF.Relu, bias=b1t[:, m : m + 1]
                        )
                    else:
                        nc.vector.tensor_scalar(
                            dst, ps[:], b1t[:, m : m + 1], 0.0,
                            op0=ALU.add, op1=ALU.max,
                        )
                    flip += 1
                    # (sa1 activation scale is folded into W1/b1 on the host)

            # ---- remaining constants (emitted after L1 so they never gate it)
            b2t = const.tile([P, 16], F32, tag="b2")
            nc.sync.dma_start(b2t[:], b2)
            b3t = const.tile([P, 16], F32, tag="b3")
            nc.sync.dma_start(b3t[:], b3)
            b41t = const.tile([P, 8], F32, tag="b41")
            nc.sync.dma_start(b41t[:], b41)
            b42t = const.tile([P, 8], F32, tag="b42")
            nc.sync.dma_start(b42t[:], b42)
            b51t = const.tile([20, 1], F32, tag="b51")
            nc.sync.dma_start(b51t[:], b51[:, None])
            b52t = const.tile([11, 1], F32, tag="b52")
            nc.sync.dma_start(b52t[:], b52[:, None])
            stdt = const.tile([P, 4], F32, tag="stdt")
            nc.sync.dma_start(stdt[:], stdb)
            meant = const.tile([P, 4], F32, tag="meant")
            nc.sync.dma_start(meant[:], meanb)
            mlt = const.tile([P, 2], F32, tag="mlt")
            nc.sync.dma_start(mlt[:], mlb)
            islt = const.tile([P, 2], F32, tag="islt")
            nc.sync.dma_start(islt[:], islb)
            esc2t = const.tile([P, 1], F32, tag="esc2t")
            nc.sync.dma_start(esc2t[:], esc2)
            esc3t = const.tile([P, 1], F32, tag="esc3t")
            nc.sync.dma_start(esc3t[:], esc3)
            esc4t = const.tile([P, 1], F32, tag="esc4t")
            nc.sync.dma_start(esc4t[:], esc4)
            halfpi = const.tile([P, 1], F32, tag="halfpi")
            nc.vector.memset(halfpi[:], HALF_PI)
            ident = const.tile([P, P], F32)
            make_identity(nc, ident[:])

            # softmax(wt) DVE chain (PE broadcast deferred until after L4)
            wtt = const.tile([1, 10], F32, tag="wtt")
            nc.sync.dma_start(wtt[:], wtv[None, :])
            mx = const.tile([1, 1], F32, tag="mx")
            nc.vector.reduce_max(mx[:, 0:1], wtt[:], axis=AX.X)
            nm = const.tile([1, 1], F32, tag="nm")
            nc.vector.tensor_scalar_mul(nm[:], mx[:], -1.0)
            ex = const.tile([1, 10], F32, tag="ex")
            nc.scalar.activation(ex[:], wtt[:], AF.Exp, bias=nm[:])
            sm = const.tile([1, 1], F32, tag="sm")
            nc.vector.reduce_sum(sm[:, 0:1], ex[:], axis=AX.X)
            inv = const.tile([1, 1], F32, tag="inv")
            nc.vector.reciprocal(inv[:], sm[:])
            wv10 = const.tile([1, 10], F32, tag="wv10")
            nc.vector.tensor_scalar_mul(wv10[:], ex[:], inv[:])
            wvp = const.tile([32, 32], F32, tag="wvp")
            nc.vector.memset(wvp[:], 0.0)
            nc.vector.tensor_copy(
                wvp[0:1, 0:20].rearrange("p (h c) -> p h c", c=2),
                wv10[:, :, None].to_broadcast([1, 10, 2]),
            )
            onesp = const.tile([32, P], F32, tag="onesp")
            nc.vector.memset(onesp[:], 0.0)
            nc.vector.memset(onesp[0:1, :], 1.0)

            # combined x51/x52 head tile (rows 0..19 = x51, 20..30 = x52)
            x5cat = tp.tile([64, B], F32, tag="x5cat")
            nc.vector.memset(x5cat[:], 0.0)

            # ---- tail part 1: geometry from x only — emitted now so the
            # Vector engine computes it underneath the L2..L5 GEMMs.
            def t3(tag, d=1):
                return tp.tile([P, CH, d], F32, tag=tag, name=tag)

            xnt = t3("xnt", 4)
            nc.sync.dma_start(xnt[:], xn)
            x0 = t3("x0", 4)
            nc.vector.tensor_mul(
                x0[:], xnt[:], stdt[:, None, :].to_broadcast([P, CH, 4])
            )
            nc.vector.tensor_add(
                x0[:], x0[:], meant[:, None, :].to_broadcast([P, CH, 4])
            )

            th = x0[:, :, 0::2]   # [P, CH, 2] angles
            wv_ = x0[:, :, 1::2]  # [P, CH, 2] angular velocities

            # range-reduce th -> rs in [-pi, pi]:  rs = th - 2pi*rint(th/2pi)
            q = t3("q", 2)
            qi = tp.tile([P, CH, 2], I32, tag="qi")
            qr = t3("qr", 2)
            rs = t3("rs", 2)
            nc.vector.tensor_scalar_mul(q[:], th, 1.0 / TWO_PI)
            nc.vector.tensor_copy(qi[:], q[:])
            nc.vector.tensor_copy(qr[:], qi[:])
            nc.vector.scalar_tensor_tensor(
                rs[:], in0=qr[:], scalar=-TWO_PI, in1=th,
                op0=ALU.mult, op1=ALU.add,
            )
            # range-reduce th + pi/2 -> rc (for cos)
            qc = t3("qc", 2)
            qci = tp.tile([P, CH, 2], I32, tag="qci")
            qcr = t3("qcr", 2)
            rc = t3("rc", 2)
            nc.vector.tensor_scalar(
                qc[:], th, 1.0 / TWO_PI, 0.25, op0=ALU.mult, op1=ALU.add
            )
            nc.vector.tensor_copy(qci[:], qc[:])
            nc.vector.tensor_copy(qcr[:], qci[:])
            nc.vector.scalar_tensor_tensor(
                rc[:], in0=qcr[:], scalar=-TWO_PI, in1=th,
                op0=ALU.mult, op1=ALU.add,
            )
            nc.vector.tensor_scalar_add(rc[:], rc[:], HALF_PI)

            sn = t3("sn", 2)
            cs = t3("cs", 2)
            nc.scalar.activation(sn[:], rs[:], AF.Sin)
            nc.scalar.activation(cs[:], rc[:], AF.Sin)

            s1, s2 = sn[:, :, 0:1], sn[:, :, 1:2]
            c1, c2 = cs[:, :, 0:1], cs[:, :, 1:2]
            w1v, w2v = wv_[:, :, 0:1], wv_[:, :, 1:2]

            px = t3("px")
            nc.vector.tensor_add(px[:], c1, c2)
            nc.vector.tensor_scalar_mul(px[:], px[:], 3.0)
            py = t3("py")
            nc.vector.tensor_add(py[:], s1, s2)
            nc.vector.tensor_scalar(py[:], py[:], 3.0, -7.0, op0=ALU.mult, op1=ALU.add)

            s1w = t3("s1w")
            nc.vector.tensor_mul(s1w[:], s1, w1v)
            s2w = t3("s2w")
            nc.vector.tensor_mul(s2w[:], s2, w2v)
            vx = t3("vx")
            nc.vector.tensor_add(vx[:], s1w[:], s2w[:])
            nc.vector.tensor_scalar_mul(vx[:], vx[:], -3.0)
            c1w = t3("c1w")
            nc.vector.tensor_mul(c1w[:], c1, w1v)
            c2w = t3("c2w")
            nc.vector.tensor_mul(c2w[:], c2, w2v)
            vy = t3("vy")
            nc.vector.tensor_add(vy[:], c1w[:], c2w[:])
            nc.vector.tensor_scalar_mul(vy[:], vy[:], 3.0)

            pxx = t3("pxx")
            nc.vector.tensor_mul(pxx[:], px[:], px[:])
            pyy = t3("pyy")
            nc.vector.tensor_mul(pyy[:], py[:], py[:])
            # barrier scaled by 16 = alpha*beta scale (4*sigmoid each)
            barrier = t3("barrier")
            nc.vector.tensor_add(barrier[:], pxx[:], pyy[:])
            nc.vector.tensor_scalar(
                barrier[:], barrier[:], 16.0, -256.0, op0=ALU.mult, op1=ALU.add
            )

            pv1 = t3("pv1")
            nc.vector.tensor_mul(pv1[:], px[:], vx[:])
            pv2 = t3("pv2")
            nc.vector.tensor_mul(pv2[:], py[:], vy[:])
            b_dot = t3("b_dot")
            nc.vector.tensor_add(b_dot[:], pv1[:], pv2[:])
            # 2 (from b_dot) * 4 (alpha+beta sigmoid scale)
            nc.vector.tensor_scalar_mul(b_dot[:], b_dot[:], 8.0)

            w1sq = t3("w1sq")
            nc.vector.tensor_mul(w1sq[:], w1v, w1v)
            w2sq = t3("w2sq")
            nc.vector.tensor_mul(w2sq[:], w2v, w2v)
            ca = t3("ca")
            nc.vector.tensor_mul(ca[:], c1, w1sq[:])
            cb = t3("cb")
            nc.vector.tensor_mul(cb[:], c2, w2sq[:])
            nc.vector.tensor_add(ca[:], ca[:], cb[:])   # c1*w1^2 + c2*w2^2
            sa = t3("sa")
            nc.vector.tensor_mul(sa[:], s1, w1sq[:])
            sb = t3("sb")
            nc.vector.tensor_mul(sb[:], s2, w2sq[:])
            nc.vector.tensor_add(sa[:], sa[:], sb[:])   # s1*w1^2 + s2*w2^2

            vxx = t3("vxx")
            nc.vector.tensor_mul(vxx[:], vx[:], vx[:])
            vyy = t3("vyy")
            nc.vector.tensor_mul(vyy[:], vy[:], vy[:])
            vsum = t3("vsum")
            nc.vector.tensor_add(vsum[:], vxx[:], vyy[:])
            nc.vector.tensor_scalar_mul(vsum[:], vsum[:], 2.0)  # 2vx^2+2vy^2

            pca = t3("pca")
            nc.vector.tensor_mul(pca[:], px[:], ca[:])
            psa = t3("psa")
            nc.vector.tensor_mul(psa[:], py[:], sa[:])
            nc.vector.tensor_add(pca[:], pca[:], psa[:])
            lf2b = t3("lf2b")
            nc.vector.scalar_tensor_tensor(
                lf2b[:], in0=pca[:], scalar=-6.0, in1=vsum[:],
                op0=ALU.mult, op1=ALU.add,
            )  # Lf2b = 2(vx^2+vy^2) - 6*(px*ca + py*sa)

            g1 = t3("g1")
            m1 = t3("m1")
            nc.vector.tensor_mul(m1[:], py[:], c1)
            m2 = t3("m2")
            nc.vector.tensor_mul(m2[:], px[:], s1)
            nc.vector.tensor_sub(g1[:], m1[:], m2[:])
            nc.vector.tensor_scalar_mul(g1[:], g1[:], 6.0)
            g2 = t3("g2")
            nc.vector.tensor_mul(m1[:], py[:], c2)
            nc.vector.tensor_mul(m2[:], px[:], s2)
            nc.vector.tensor_sub(g2[:], m1[:], m2[:])
            nc.vector.tensor_scalar_mul(g2[:], g2[:], 6.0)

            gdot = t3("gdot")
            g1sq = t3("g1sq")
            nc.vector.tensor_mul(g1sq[:], g1[:], g1[:])
            g2sq = t3("g2sq")
            nc.vector.tensor_mul(g2sq[:], g2[:], g2[:])
            nc.vector.tensor_add(gdot[:], g1sq[:], g2sq[:])
            igdot = t3("igdot")
            nc.vector.reciprocal(igdot[:], gdot[:])

            # ---- generic streamed GEMM layer ----
            def mlp_layer(wdram, KT, MT, MD, hin, kin_base, btile, evict,
                          prefetched=None, dr=False):
                wdt = FP8 if dr else BF16
                for m in range(MT):
                    mp = min(P, MD - m * P)
                    if prefetched is not None and m in prefetched:
                        wcol = prefetched[m]
                    else:
                        wcol = wpool.tile([P, KT, mp], wdt, tag="wcol")
                        if len(wdram.shape) == 4:
                            nc.sync.dma_start(wcol[:], wdram[:, m])
                        else:
                            nc.sync.dma_start(wcol[:], wdram)
                    for g in range(NB // 2):
                        ps = psum.tile([P, 2 * NF], F32, tag="mm")
                        for half in range(2):
                            n = 2 * g + half
                            if dr:
                                for k2 in range(KT // 2):
                                    nc.tensor.matmul(
                                        ps[:mp, half * NF : (half + 1) * NF],
                                        wcol[:, 2 * k2 : 2 * k2 + 2, :],
                                        hin[:, kin_base + 2 * k2 : kin_base + 2 * k2 + 2,
                                            n * NF : (n + 1) * NF],
                                        start=(k2 == 0),
                                        stop=(k2 == KT // 2 - 1),
                                        perf_mode=mybir.MatmulPerfMode.DoubleRow,
                                    )
                            else:
                                for k in range(KT):
                                    nc.tensor.matmul(
                                        ps[:mp, half * NF : (half + 1) * NF],
                                        wcol[:, k, :],
                                        hin[:, kin_base + k, n * NF : (n + 1) * NF],
                                        start=(k == 0),
                                        stop=(k == KT - 1),
                                    )
                        evict(m, g, ps[:mp])

            # ---- L2 / L3 / L4 ----
            h2 = hpool.tile([P, 16, B], h2dt, tag="act", name="h2")

            def ev_h(hout, btile, m_off=0, scale=1.0):
                def _e(m, g, ps):
                    nc.scalar.activation(
                        hout[:, m_off + m, 2 * g * NF : 2 * (g + 1) * NF], ps,
                        AF.Relu, bias=btile[:, m : m + 1], scale=scale,
                    )
                return _e

            sc2 = esc2t[:]
            mlp_layer(w2, 16, 16, 2048, h1, 0, b2t, ev_h(h2, b2t, 0, sc2),
                      prefetched={0: wcol2_0}, dr="l2" in fp8_layers)

            h3 = hpool.tile([P, 16, B], h3dt, tag="act", name="h3")
            sc3 = esc3t[:]
            mlp_layer(w3, 16, 16, 2048, h2, 0, b3t, ev_h(h3, b3t, 0, sc3),
                      dr="l3" in fp8_layers)

            h4 = hpool.tile([P, 16, B], BF16, tag="act", name="h4")
            sc4 = esc4t[:]
            mlp_layer(w41, 8, 8, 1024, h3, 0, b41t, ev_h(h4, b41t, 0, sc4),
                      dr="l4" in fp8_layers)
            mlp_layer(w42, 8, 8, 1024, h3, 8, b42t, ev_h(h4, b42t, 8, sc4),
                      dr="l4" in fp8_layers)

            # ---- L5 into the combined head tile ----
            def ev_51(m, g, ps):
                nc.vector.tensor_scalar_add(
                    x5cat[:20, 2 * g * NF : 2 * (g + 1) * NF], ps, b51t[:]
                )

            def ev_52(m, g, ps):
                nc.scalar.activation(
                    x5cat[32:43, 2 * g * NF : 2 * (g + 1) * NF], ps, AF.Sigmoid,
                    bias=b52t[:],
                )

            mlp_layer(w51, 8, 1, 20, h4, 0, b51t, ev_51, prefetched={0: w51c})
            mlp_layer(w52, 8, 1, 11, h4, 8, b52t, ev_52, prefetched={0: w52c})

            # wv broadcast to all partitions (PE hits this after L4/L5)
            pwv = psum.tile([P, 2 * NF], F32, tag="mm", name="pwv")
            nc.tensor.matmul(pwv[:, :32], onesp[:], wvp[:], start=True, stop=True)
            wv20 = const.tile([P, 20], F32, tag="wv20")
            nc.vector.tensor_copy(wv20[:], pwv[:, :20])

            # ---- transpose the combined head tile: 16 chunks of [32,128] ----
            x5t = tp.tile([P, CH, 43], F32, tag="x5t")
            for c in range(CH):
                pt = psum.tile([P, 2 * NF], F32, tag="mm", name="pt")
                nc.tensor.transpose(
                    pt[:, :64], x5cat[:, c * P : (c + 1) * P], ident[:64, :64]
                )
                if c % 2 == 0:
                    nc.vector.tensor_copy(x5t[:, c, :], pt[:, :43])
                else:
                    nc.scalar.copy(x5t[:, c, :], pt[:, :43])

            # ---- tail part 2: per-head analytic QP ----
            B10 = [P, CH, 10]
            alpha = x5t[:, :, 32:33]
            betas = x5t[:, :, 33:43]
            apb = t3("apb", 10)
            nc.vector.tensor_add(apb[:], betas, alpha.to_broadcast(B10))
            ab = t3("ab", 10)
            nc.vector.tensor_mul(ab[:], betas, alpha.to_broadcast(B10))
            hv = t3("hv", 10)
            nc.vector.tensor_mul(hv[:], apb[:], b_dot[:].to_broadcast(B10))
            hv2 = t3("hv2", 10)
            nc.vector.tensor_mul(hv2[:], ab[:], barrier[:].to_broadcast(B10))
            nc.vector.tensor_add(hv[:], hv[:], hv2[:])
            nc.vector.tensor_add(hv[:], hv[:], lf2b[:].to_broadcast(B10))

            refx = x5t[:, :, 0:20:2]
            refy = x5t[:, :, 1:20:2]
            viol = t3("viol", 10)
            v2t = t3("v2t", 10)
            nc.vector.tensor_mul(viol[:], refx, g1[:].to_broadcast(B10))
            nc.vector.tensor_mul(v2t[:], refy, g2[:].to_broadcast(B10))
            nc.vector.tensor_add(viol[:], viol[:], v2t[:])
            nc.vector.tensor_sub(viol[:], viol[:], hv[:])

            lam = t3("lam", 10)
            nc.vector.tensor_mul(lam[:], viol[:], igdot[:].to_broadcast(B10))
            nc.vector.tensor_scalar_max(lam[:], lam[:], 0.0)

            # S = sum_h wv_h * lam_h ; refbar = sum_h wv_h * ref_h
            wlam = t3("wlam", 10)
            nc.vector.tensor_mul(
                wlam[:], lam[:], wv20[:, None, 0::2].to_broadcast(B10)
            )
            S = t3("S")
            nc.vector.reduce_sum(S[:, :, 0], wlam[:], axis=AX.X)

            wref = t3("wref", 20)
            nc.vector.tensor_mul(
                wref[:], x5t[:, :, 0:20], wv20[:, None, :].to_broadcast([P, CH, 20])
            )
            rbx = t3("rbx")
            nc.vector.reduce_sum(rbx[:, :, 0], wref[:, :, 0::2], axis=AX.X)
            rby = t3("rby")
            nc.vector.reduce_sum(rby[:, :, 0], wref[:, :, 1::2], axis=AX.X)

            rtx = t3("rtx")
            nc.vector.tensor_mul(rtx[:], S[:], g1[:])
            nc.vector.tensor_sub(rtx[:], rtx[:], rbx[:])
            rty = t3("rty")
            nc.vector.tensor_mul(rty[:], S[:], g2[:])
            nc.vector.tensor_sub(rty[:], rty[:], rby[:])

            ot = t3("ot", 2)
            nc.vector.tensor_scalar(
                ot[:, :, 0], rtx[:, :, 0], mlt[:, 0:1], islt[:, 0:1],
                op0=ALU.subtract, op1=ALU.mult,
            )
            nc.vector.tensor_scalar(
                ot[:, :, 1], rty[:, :, 0], mlt[:, 1:2], islt[:, 1:2],
                op0=ALU.subtract, op1=ALU.mult,
            )
            nc.sync.dma_start(out, ot[:])

    nc.compile()
    return nc


def configure(fp8_layers):
    """Select fp8 layers; must be called before the first kernel() call."""
    global FP8_LAYERS, _CACHED_NC
    if frozenset(fp8_layers) != FP8_LAYERS:
        FP8_LAYERS = frozenset(fp8_layers)
        _CACHED_NC = None


def _get_nc():
    global _CACHED_NC
    if _CACHED_NC is None:
        _CACHED_NC = _build(FP8_LAYERS)
    return _CACHED_NC


def _bf16(a):
    return np.ascontiguousarray(a.astype(ml_dtypes.bfloat16))


def _f32(a):
    return np.ascontiguousarray(np.asarray(a, dtype=np.float32))


def _e4(a):
    dt = mybir.dt.np(mybir.dt.float8e4)
    return np.ascontiguousarray(a.astype(dt))


def _prep_inputs(inputs):
    x = _f32(inputs["x"])
    mean = _f32(inputs["mean"])
    std = _f32(inputs["std"])
    mean_label = _f32(inputs["mean_label"])
    std_label = _f32(inputs["std_label"])
    wt = _f32(inputs["wt"])
    W1, b1 = _f32(inputs["W1"]), _f32(inputs["b1"])
    W2, b2 = _f32(inputs["W2"]), _f32(inputs["b2"])
    W31, b31 = _f32(inputs["W31"]), _f32(inputs["b31"])
    W32, b32 = _f32(inputs["W32"]), _f32(inputs["b32"])
    W41, b41 = _f32(inputs["W41"]), _f32(inputs["b41"])
    W42, b42 = _f32(inputs["W42"]), _f32(inputs["b42"])
    W51, b51 = _f32(inputs["W51"]), _f32(inputs["b51"])
    W52, b52 = _f32(inputs["W52"]), _f32(inputs["b52"])

    def pack4(wT, KT, MT):  # (K, M) -> (128, MT, KT, 128)
        return _bf16(wT.reshape(KT, P, MT, P).transpose(1, 2, 0, 3))

    def pack3(wT, KT):  # (K, M) -> (128, KT, M)
        K, M = wT.shape
        return _bf16(wT.reshape(KT, P, M).transpose(1, 0, 2))

    W3T = np.concatenate([W31.T, W32.T], axis=1)  # (2048, 2048)
    b3 = np.concatenate([b31, b32])

    # fp8 calibration: static activation scales from a row subsample
    # (4.7x headroom to the e4m3 max), per-tensor weight scales.
    sa1 = sa2 = sa3 = 1.0
    sw2 = sw3 = sw4 = 1.0
    if FP8_LAYERS:
        xs = x[:512]
        h1s = np.maximum(xs @ W1.T + b1, 0.0)
        if "l2" in FP8_LAYERS:
            sa1 = 96.0 / max(float(np.abs(h1s).max()), 1e-30)
            sw2 = 192.0 / max(float(np.abs(W2).max()), 1e-30)
        if "l3" in FP8_LAYERS or "l4" in FP8_LAYERS:
            h2s = np.maximum(h1s @ W2.T + b2, 0.0)
            if "l3" in FP8_LAYERS:
                sa2 = 96.0 / max(float(np.abs(h2s).max()), 1e-30)
                sw3 = 192.0 / max(float(np.abs(W3T).max()), 1e-30)
            if "l4" in FP8_LAYERS:
                h3s = np.maximum(h2s @ W3T + b3, 0.0)
                sa3 = 96.0 / max(float(np.abs(h3s).max()), 1e-30)
                sw4 = 192.0 / max(
                    float(max(np.abs(W41).max(), np.abs(W42).max())), 1e-30
                )
    esc2 = sa2 / (sw2 * sa1)
    esc3 = sa3 / (sw3 * sa2)
    esc4 = 1.0 / (sw4 * sa3)

    def wpack(wT, KT, MT, sw, fp8):
        packed = wT.reshape(KT, P, MT, P).transpose(1, 2, 0, 3)
        if fp8:
            return _e4(packed * sw)
        return _bf16(packed)

    w1p = np.zeros((P, 2048), np.float32)
    w1p[:4] = W1.T
    shared = {
        "w1": _bf16(w1p * sa1),
        "w2": wpack(W2.T, 16, 16, sw2, "l2" in FP8_LAYERS),
        "w3": wpack(W3T, 16, 16, sw3, "l3" in FP8_LAYERS),
        "w41": wpack(W41.T, 8, 8, sw4, "l4" in FP8_LAYERS),
        "w42": wpack(W42.T, 8, 8, sw4, "l4" in FP8_LAYERS),
        "w51": pack3(W51.T, 8),
        "w52": pack3(W52.T, 8),
        "esc2": _f32(np.full((P, 1), esc2)),
        "esc3": _f32(np.full((P, 1), esc3)),
        "esc4": _f32(np.full((P, 1), esc4)),
        "b1": _f32(b1.reshape(16, P).T * sa1),
        "b2": _f32(b2.reshape(16, P).T * sa2),
        "b3": _f32(b3.reshape(16, P).T * sa3),
        "b41": _f32(b41.reshape(8, P).T),
        "b42": _f32(b42.reshape(8, P).T),
        "b51": b51,
        "b52": b52,
        "stdb": _f32(np.tile(std[None, :], (P, 1))),
        "meanb": _f32(np.tile(mean[None, :], (P, 1))),
        "mlb": _f32(np.tile(mean_label[None, :], (P, 1))),
        "islb": _f32(np.tile((1.0 / std_label)[None, :], (P, 1))),
        "wtv": wt,
    }

    in_maps = []
    for i in range(N_CORES):
        xs = x[i * B : (i + 1) * B]  # (2048, 4)
        m = dict(shared)
        xtp = np.zeros((P, B), np.float32)
        xtp[:4] = xs.T
        m["xt"] = _bf16(xtp)
        m["xn"] = _f32(xs.reshape(CH, P, 4).transpose(1, 0, 2))
        in_maps.append(m)
    return in_maps


def kernel_core(inputs, trace=False):
    nc = _get_nc()
    in_maps = _prep_inputs(inputs)
    res = run_bass_kernel_spmd(
        nc, in_maps, core_ids=list(range(N_CORES)), trace=trace
    )
    shards = []
    for i in range(N_CORES):
        o = res.results[i]["out"]  # (128, 16, 2)
        shards.append(o.transpose(1, 0, 2).reshape(B, 2))
    return np.concatenate(shards, axis=0).astype(np.float32), res


def kernel(**inputs):
    out, _ = kernel_core(inputs)
    return out


# revision 24
# speedup vs baseline: 2.1962x; 1.1984x over previous
"""Trainium2 Bass kernel for nn_ABNet_U (multi-branch MLP + CBF-QP head).

Data-parallel over batch: 16384 rows -> 8 NeuronCores x 2048 rows, weights
replicated and host-prepped into K-major layouts.  The three large middle
GEMMs (L2/L3/L4) run in fp8-e4m3 with DoubleRow perf mode (2 weights per
PE cell -> half the matmul count); L1 and the small heads stay bf16.  All
GEMMs accumulate in fp32 PSUM with fused scale+bias+activation eviction on
the Scalar/Vector engines (fp8 rescales fold into the eviction scale).
The trig/barrier/QP tail runs in fp32 on the Vector engine with batch on
partitions, emitted early so it executes underneath the GEMM phase.
Measured: rel err 1.40e-2 vs the fp32 reference (gate 2e-2), ~336us/core
at full PE clock.  Set FP8_LAYERS = frozenset() for the bf16-exact
fallback (rel err 9.3e-4, ~612us).
"""

import sys

sys.path.insert(0, "/opt/trn_rl_repo")

import numpy as np
import ml_dtypes

import concourse.bass as bass
import concourse.mybir as mybir
import concourse.tile as tile
from concourse import bacc
from concourse.bass_utils import run_bass_kernel_spmd
from concourse.masks import make_identity

N_CORES = 8
B_GLOBAL = 16384
B = B_GLOBAL // N_CORES  # 2048 rows per core
P = 128
CH = B // P              # 16 batch chunks of 128 (tail layout)
NF = 512                 # matmul free-dim chunk
NB = B // NF             # 4 free chunks
HEADS = 10

AF = mybir.ActivationFunctionType
ALU = mybir.AluOpType
AX = mybir.AxisListType
F32 = mybir.dt.float32
BF16 = mybir.dt.bfloat16
I32 = mybir.dt.int32

TWO_PI = float(2.0 * np.pi)
HALF_PI = float(0.5 * np.pi)

# Layers computed in fp8-e4m3 with DoubleRow (2 weights/PE cell, halves the
# matmul count).  Weights use per-tensor scales; activations use static
# per-layer scales calibrated host-side on a row subsample with 4.7x
# headroom to the e4m3 max.
FP8_LAYERS = frozenset({"l2", "l3", "l4"})  # override via configure()
FP8 = mybir.dt.float8e4

_CACHED_NC = None


def _build(fp8_layers=frozenset()):
    nc = bacc.Bacc(
        "TRN2",
        target_bir_lowering=False,
        debug=False,
        enable_asserts=False,
        num_devices=N_CORES,
    )

    def din(name, shape, dt=F32):
        return nc.dram_tensor(name, list(shape), dt, kind="ExternalInput").ap()

    xt = din("xt", (P, B), BF16)
    xn = din("xn", (P, CH, 4))              # x shard, [p, chunk, feat] fp32
    w1 = din("w1", (P, 2048), BF16)
    w2 = din("w2", (P, 16, 16, P), FP8 if "l2" in fp8_layers else BF16)
    w3 = din("w3", (P, 16, 16, P), FP8 if "l3" in fp8_layers else BF16)
    w41 = din("w41", (P, 8, 8, P), FP8 if "l4" in fp8_layers else BF16)
    w42 = din("w42", (P, 8, 8, P), FP8 if "l4" in fp8_layers else BF16)
    esc2 = din("esc2", (P, 1))
    esc3 = din("esc3", (P, 1))
    esc4 = din("esc4", (P, 1))
    w51 = din("w51", (P, 8, 20), BF16)      # [p, kt, m]
    w52 = din("w52", (P, 8, 11), BF16)
    b1 = din("b1", (P, 16))
    b2 = din("b2", (P, 16))
    b3 = din("b3", (P, 16))
    b41 = din("b41", (P, 8))
    b42 = din("b42", (P, 8))
    b51 = din("b51", (20,))
    b52 = din("b52", (11,))
    stdb = din("stdb", (P, 4))
    meanb = din("meanb", (P, 4))
    mlb = din("mlb", (P, 2))
    islb = din("islb", (P, 2))
    wtv = din("wtv", (10,))
    out = nc.dram_tensor("out", [P, CH, 2], F32, kind="ExternalOutput").ap()

    with tile.TileContext(nc) as tc:
        from contextlib import ExitStack

        with ExitStack() as ctx:
            const = ctx.enter_context(tc.tile_pool(name="const", bufs=1))
            wpool = ctx.enter_context(tc.tile_pool(name="wpool", bufs=3))
            hpool = ctx.enter_context(tc.tile_pool(name="hpool", bufs=2))
            psum = ctx.enter_context(tc.tile_pool(name="psum", bufs=4, space="PSUM"))
            tp = ctx.enter_context(tc.tile_pool(name="tp", bufs=1))

            # ---- L1-critical loads first: keep the PE fed from t=0 ----
            b1t = const.tile([P, 16], F32, tag="b1")
            nc.sync.dma_start(b1t[:], b1)
            xtb = const.tile([P, B], BF16, tag="xtb")
            nc.sync.dma_start(xtb[:], xt)
            w1tb = const.tile([P, 2048], BF16, tag="w1tb")
            nc.sync.dma_start(w1tb[:], w1)

            # prefetch: L2's first weight column gates the L1->L2 handoff;
            # the tiny L5 weight tiles ride the otherwise-idle gpsimd queue.
            wcol2_0 = wpool.tile(
                [P, 16, P], FP8 if "l2" in fp8_layers else BF16,
                tag="wcol", name="wcol2_0",
            )
            nc.sync.dma_start(wcol2_0[:], w2[:, 0])
            w51c = const.tile([P, 8, 20], BF16, tag="w51c")
            nc.gpsimd.dma_start(w51c[:], w51)
            w52c = const.tile([P, 8, 11], BF16, tag="w52c")
            nc.gpsimd.dma_start(w52c[:], w52)

            # PE warm-up: dummy matmuls fill the idle window while the L1
            # input DMAs land, and push the HAM clock gate to 8/8 before the
            # real matmul stream begins.
            wrm = const.tile([P, NF], BF16, tag="wrm")
            nc.vector.memset(wrm[:], 0.0)
            wps = psum.tile([P, 2 * NF], F32, tag="mm", name="wps")
            for _ in range(20):
                nc.tensor.matmul(
                    wps[:, :NF], wrm[:, :P], wrm[:], start=True, stop=True
                )

            # warm the ACT engine's function tables in the startup window so
            # the 1.3us table loads don't land mid-stream later
            tbl = const.tile([1, 2], F32, tag="tbl")
            nc.vector.memset(tbl[:], 0.25)
            nc.scalar.activation(tbl[:, 1:2], tbl[:, 0:1], AF.Relu)
            nc.scalar.activation(tbl[:, 1:2], tbl[:, 0:1], AF.Exp)
            nc.scalar.activation(tbl[:, 1:2], tbl[:, 0:1], AF.Sin)
            nc.scalar.activation(tbl[:, 1:2], tbl[:, 0:1], AF.Sigmoid)
            nc.scalar.activation(tbl[:, 1:2], tbl[:, 0:1], AF.Identity)

            # ---- L1: h1 = relu(W1 @ x^T + b1), K=4 zero-padded to 128 ----
            # One matmul per eviction, so evictions bound this layer: split
            # them across the Scalar and Vector engines.
            h1dt = FP8 if "l2" in fp8_layers else BF16
            h2dt = FP8 if "l3" in fp8_layers else BF16
            h3dt = FP8 if "l4" in fp8_layers else BF16
            h1 = hpool.tile([P, 16, B], h1dt, tag="act", name="h1")
            flip = 0
            for g in range(NB // 2):
                for m in range(16):
                    ps = psum.tile([P, 2 * NF], F32, tag="mm")
                    for half in range(2):
                        n = 2 * g + half
                        nc.tensor.matmul(
                            ps[:, half * NF : (half + 1) * NF],
                            w1tb[:, m * P : (m + 1) * P],
                            xtb[:, n * NF : (n + 1) * NF],
                            start=True,
                            stop=True,
                        )
                    dst = h1[:, m, 2 * g * NF : 2 * (g + 1) * NF]
                    if flip % 2 == 0:
                        nc.scalar.activation(
                            dst, ps[:], AF.Relu, bias=b1t[:, m : m + 1]
                        )
                    else:
                        nc.vector.tensor_scalar(
                            dst, ps[:], b1t[:, m : m + 1], 0.0,
                            op0=ALU.add, op1=ALU.max,
                        )
                    flip += 1
                    # (sa1 activation scale is folded into W1/b1 on the host)

            # ---- remaining constants (emitted after L1 so they never gate it)
            b2t = const.tile([P, 16], F32, tag="b2")
            nc.sync.dma_start(b2t[:], b2)
            b3t = const.tile([P, 16], F32, tag="b3")
            nc.sync.dma_start(b3t[:], b3)
            b41t = const.tile([P, 8], F32, tag="b41")
            nc.sync.dma_start(b41t[:], b41)
            b42t = const.tile([P, 8], F32, tag="b42")
            nc.sync.dma_start(b42t[:], b42)
            b51t = const.tile([20, 1], F32, tag="b51")
            nc.sync.dma_start(b51t[:], b51[:, None])
            b52t = const.tile([11, 1], F32, tag="b52")
            nc.sync.dma_start(b52t[:], b52[:, None])
            stdt = const.tile([P, 4], F32, tag="stdt")
            nc.sync.dma_start(stdt[:], stdb)
            meant = const.tile([P, 4], F32, tag="meant")
            nc.sync.dma_start(meant[:], meanb)
            mlt = const.tile([P, 2], F32, tag="mlt")
            nc.sync.dma_start(mlt[:], mlb)
            islt = const.tile([P, 2], F32, tag="islt")
            nc.sync.dma_start(islt[:], islb)
            esc2t = const.tile([P, 1], F32, tag="esc2t")
            nc.sync.dma_start(esc2t[:], esc2)
            esc3t = const.tile([P, 1], F32, tag="esc3t")
            nc.sync.dma_start(esc3t[:], esc3)
            esc4t = const.tile([P, 1], F32, tag="esc4t")
            nc.sync.dma_start(esc4t[:], esc4)
            halfpi = const.tile([P, 1], F32, tag="halfpi")
            nc.vector.memset(halfpi[:], HALF_PI)
            ident = const.tile([P, P], F32)
            make_identity(nc, ident[:])

            # softmax(wt) DVE chain (PE broadcast deferred until after L4)
            wtt = const.tile([1, 10], F32, tag="wtt")
            nc.sync.dma_start(wtt[:], wtv[None, :])
            mx = const.tile([1, 1], F32, tag="mx")
            nc.vector.reduce_max(mx[:, 0:1], wtt[:], axis=AX.X)
            nm = const.tile([1, 1], F32, tag="nm")
            nc.vector.tensor_scalar_mul(nm[:], mx[:], -1.0)
            ex = const.tile([1, 10], F32, tag="ex")
            nc.scalar.activation(ex[:], wtt[:], AF.Exp, bias=nm[:])
            sm = const.tile([1, 1], F32, tag="sm")
            nc.vector.reduce_sum(sm[:, 0:1], ex[:], axis=AX.X)
            inv = const.tile([1, 1], F32, tag="inv")
            nc.vector.reciprocal(inv[:], sm[:])
            wv10 = const.tile([1, 10], F32, tag="wv10")
            nc.vector.tensor_scalar_mul(wv10[:], ex[:], inv[:])
            wvp = const.tile([32, 32], F32, tag="wvp")
            nc.vector.memset(wvp[:], 0.0)
            nc.vector.tensor_copy(
                wvp[0:1, 0:20].rearrange("p (h c) -> p h c", c=2),
                wv10[:, :, None].to_broadcast([1, 10, 2]),
            )
            onesp = const.tile([32, P], F32, tag="onesp")
            nc.vector.memset(onesp[:], 0.0)
            nc.vector.memset(onesp[0:1, :], 1.0)

            # combined x51/x52 head tile (rows 0..19 = x51, 20..30 = x52)
            x5cat = tp.tile([64, B], F32, tag="x5cat")
            nc.vector.memset(x5cat[:], 0.0)

            # ---- tail part 1: geometry from x only — emitted now so the
            # Vector engine computes it underneath the L2..L5 GEMMs.
            def t3(tag, d=1):
                return tp.tile([P, CH, d], F32, tag=tag, name=tag)

            xnt = t3("xnt", 4)
            nc.sync.dma_start(xnt[:], xn)
            x0 = t3("x0", 4)
            nc.vector.tensor_mul(
                x0[:], xnt[:], stdt[:, None, :].to_broadcast([P, CH, 4])
            )
            nc.vector.tensor_add(
                x0[:], x0[:], meant[:, None, :].to_broadcast([P, CH, 4])
            )

            th = x0[:, :, 0::2]   # [P, CH, 2] angles
            wv_ = x0[:, :, 1::2]  # [P, CH, 2] angular velocities

            # range-reduce th -> rs in [-pi, pi]:  rs = th - 2pi*rint(th/2pi)
            q = t3("q", 2)
            qi = tp.tile([P, CH, 2], I32, tag="qi")
            qr = t3("qr", 2)
            rs = t3("rs", 2)
            nc.vector.tensor_scalar_mul(q[:], th, 1.0 / TWO_PI)
            nc.vector.tensor_copy(qi[:], q[:])
            nc.vector.tensor_copy(qr[:], qi[:])
            nc.vector.scalar_tensor_tensor(
                rs[:], in0=qr[:], scalar=-TWO_PI, in1=th,
                op0=ALU.mult, op1=ALU.add,
            )
            # range-reduce th + pi/2 -> rc (for cos)
            qc = t3("qc", 2)
            qci = tp.tile([P, CH, 2], I32, tag="qci")
            qcr = t3("qcr", 2)
            rc = t3("rc", 2)
            nc.vector.tensor_scalar(
                qc[:], th, 1.0 / TWO_PI, 0.25, op0=ALU.mult, op1=ALU.add
            )
            nc.vector.tensor_copy(qci[:], qc[:])
            nc.vector.tensor_copy(qcr[:], qci[:])
            nc.vector.scalar_tensor_tensor(
                rc[:], in0=qcr[:], scalar=-TWO_PI, in1=th,
                op0=ALU.mult, op1=ALU.add,
            )
            nc.vector.tensor_scalar_add(rc[:], rc[:], HALF_PI)

            sn = t3("sn", 2)
            cs = t3("cs", 2)
            nc.scalar.activation(sn[:], rs[:], AF.Sin)
            nc.scalar.activation(cs[:], rc[:], AF.Sin)

            s1, s2 = sn[:, :, 0:1], sn[:, :, 1:2]
            c1, c2 = cs[:, :, 0:1], cs[:, :, 1:2]
            w1v, w2v = wv_[:, :, 0:1], wv_[:, :, 1:2]

            px = t3("px")
            nc.vector.tensor_add(px[:], c1, c2)
            nc.vector.tensor_scalar_mul(px[:], px[:], 3.0)
            py = t3("py")
            nc.vector.tensor_add(py[:], s1, s2)
            nc.vector.tensor_scalar(py[:], py[:], 3.0, -7.0, op0=ALU.mult, op1=ALU.add)

            s1w = t3("s1w")
            nc.vector.tensor_mul(s1w[:], s1, w1v)
            s2w = t3("s2w")
            nc.vector.tensor_mul(s2w[:], s2, w2v)
            vx = t3("vx")
            nc.vector.tensor_add(vx[:], s1w[:], s2w[:])
            nc.vector.tensor_scalar_mul(vx[:], vx[:], -3.0)
            c1w = t3("c1w")
            nc.vector.tensor_mul(c1w[:], c1, w1v)
            c2w = t3("c2w")
            nc.vector.tensor_mul(c2w[:], c2, w2v)
            vy = t3("vy")
            nc.vector.tensor_add(vy[:], c1w[:], c2w[:])
            nc.vector.tensor_scalar_mul(vy[:], vy[:], 3.0)

            pxx = t3("pxx")
            nc.vector.tensor_mul(pxx[:], px[:], px[:])
            pyy = t3("pyy")
            nc.vector.tensor_mul(pyy[:], py[:], py[:])
            # barrier scaled by 16 = alpha*beta scale (4*sigmoid each)
            barrier = t3("barrier")
            nc.vector.tensor_add(barrier[:], pxx[:], pyy[:])
            nc.vector.tensor_scalar(
                barrier[:], barrier[:], 16.0, -256.0, op0=ALU.mult, op1=ALU.add
            )

            pv1 = t3("pv1")
            nc.vector.tensor_mul(pv1[:], px[:], vx[:])
            pv2 = t3("pv2")
            nc.vector.tensor_mul(pv2[:], py[:], vy[:])
            b_dot = t3("b_dot")
            nc.vector.tensor_add(b_dot[:], pv1[:], pv2[:])
            # 2 (from b_dot) * 4 (alpha+beta sigmoid scale)
            nc.vector.tensor_scalar_mul(b_dot[:], b_dot[:], 8.0)

            w1sq = t3("w1sq")
            nc.vector.tensor_mul(w1sq[:], w1v, w1v)
            w2sq = t3("w2sq")
            nc.vector.tensor_mul(w2sq[:], w2v, w2v)
            ca = t3("ca")
            nc.vector.tensor_mul(ca[:], c1, w1sq[:])
            cb = t3("cb")
            nc.vector.tensor_mul(cb[:], c2, w2sq[:])
            nc.vector.tensor_add(ca[:], ca[:], cb[:])   # c1*w1^2 + c2*w2^2
            sa = t3("sa")
            nc.vector.tensor_mul(sa[:], s1, w1sq[:])
            sb = t3("sb")
            nc.vector.tensor_mul(sb[:], s2, w2sq[:])
            nc.vector.tensor_add(sa[:], sa[:], sb[:])   # s1*w1^2 + s2*w2^2

            vxx = t3("vxx")
            nc.vector.tensor_mul(vxx[:], vx[:], vx[:])
            vyy = t3("vyy")
            nc.vector.tensor_mul(vyy[:], vy[:], vy[:])
            vsum = t3("vsum")
            nc.vector.tensor_add(vsum[:], vxx[:], vyy[:])
            nc.vector.tensor_scalar_mul(vsum[:], vsum[:], 2.0)  # 2vx^2+2vy^2

            pca = t3("pca")
            nc.vector.tensor_mul(pca[:], px[:], ca[:])
            psa = t3("psa")
            nc.vector.tensor_mul(psa[:], py[:], sa[:])
            nc.vector.tensor_add(pca[:], pca[:], psa[:])
            lf2b = t3("lf2b")
            nc.vector.scalar_tensor_tensor(
                lf2b[:], in0=pca[:], scalar=-6.0, in1=vsum[:],
                op0=ALU.mult, op1=ALU.add,
            )  # Lf2b = 2(vx^2+vy^2) - 6*(px*ca + py*sa)

            g1 = t3("g1")
            m1 = t3("m1")
            nc.vector.tensor_mul(m1[:], py[:], c1)
            m2 = t3("m2")
            nc.vector.tensor_mul(m2[:], px[:], s1)
            nc.vector.tensor_sub(g1[:], m1[:], m2[:])
            nc.vector.tensor_scalar_mul(g1[:], g1[:], 6.0)
            g2 = t3("g2")
            nc.vector.tensor_mul(m1[:], py[:], c2)
            nc.vector.tensor_mul(m2[:], px[:], s2)
            nc.vector.tensor_sub(g2[:], m1[:], m2[:])
            nc.vector.tensor_scalar_mul(g2[:], g2[:], 6.0)

            gdot = t3("gdot")
            g1sq = t3("g1sq")
            nc.vector.tensor_mul(g1sq[:], g1[:], g1[:])
            g2sq = t3("g2sq")
            nc.vector.tensor_mul(g2sq[:], g2[:], g2[:])
            nc.vector.tensor_add(gdot[:], g1sq[:], g2sq[:])
            igdot = t3("igdot")
            nc.vector.reciprocal(igdot[:], gdot[:])

            # ---- generic streamed GEMM layer ----
            def mlp_layer(wdram, KT, MT, MD, hin, kin_base, btile, evict,
                          prefetched=None, dr=False):
                wdt = FP8 if dr else BF16
                for m in range(MT):
                    mp = min(P, MD - m * P)
                    if prefetched is not None and m in prefetched:
                        wcol = prefetched[m]
                    else:
                        wcol = wpool.tile([P, KT, mp], wdt, tag="wcol")
                        if len(wdram.shape) == 4:
                            nc.sync.dma_start(wcol[:], wdram[:, m])
                        else:
                            nc.sync.dma_start(wcol[:], wdram)
                    for g in range(NB // 2):
                        ps = psum.tile([P, 2 * NF], F32, tag="mm")
                        for half in range(2):
                            n = 2 * g + half
                            if dr:
                                for k2 in range(KT // 2):
                                    nc.tensor.matmul(
                                        ps[:mp, half * NF : (half + 1) * NF],
                                        wcol[:, 2 * k2 : 2 * k2 + 2, :],
                                        hin[:, kin_base + 2 * k2 : kin_base + 2 * k2 + 2,
                                            n * NF : (n + 1) * NF],
                                        start=(k2 == 0),
                                        stop=(k2 == KT // 2 - 1),
                                        perf_mode=mybir.MatmulPerfMode.DoubleRow,
                                    )
                            else:
                                for k in range(KT):
                                    nc.tensor.matmul(
                                        ps[:mp, half * NF : (half + 1) * NF],
                                        wcol[:, k, :],
                                        hin[:, kin_base + k, n * NF : (n + 1) * NF],
                                        start=(k == 0),
                                        stop=(k == KT - 1),
                                    )
                        evict(m, g, ps[:mp])

            # ---- L2 / L3 / L4 ----
            h2 = hpool.tile([P, 16, B], h2dt, tag="act", name="h2")

            def ev_h(hout, btile, m_off=0, scale=1.0):
                def _e(m, g, ps):
                    nc.scalar.activation(
                        hout[:, m_off + m, 2 * g * NF : 2 * (g + 1) * NF], ps,
                        AF.Relu, bias=btile[:, m : m + 1], scale=scale,
                    )
                return _e

            sc2 = esc2t[:]
            mlp_layer(w2, 16, 16, 2048, h1, 0, b2t, ev_h(h2, b2t, 0, sc2),
                      prefetched={0: wcol2_0}, dr="l2" in fp8_layers)

            h3 = hpool.tile([P, 16, B], h3dt, tag="act", name="h3")
            sc3 = esc3t[:]
            mlp_layer(w3, 16, 16, 2048, h2, 0, b3t, ev_h(h3, b3t, 0, sc3),
                      dr="l3" in fp8_layers)

            h4 = hpool.tile([P, 16, B], BF16, tag="act", name="h4")
            sc4 = esc4t[:]
            mlp_layer(w41, 8, 8, 1024, h3, 0, b41t, ev_h(h4, b41t, 0, sc4),
                      dr="l4" in fp8_layers)
            mlp_layer(w42, 8, 8, 1024, h3, 8, b42t, ev_h(h4, b42t, 8, sc4),
                      dr="l4" in fp8_layers)

            # ---- L5 into the combined head tile ----
            def ev_51(m, g, ps):
                nc.vector.tensor_scalar_add(
                    x5cat[:20, 2 * g * NF : 2 * (g + 1) * NF], ps, b51t[:]
                )

            def ev_52(m, g, ps):
                nc.scalar.activation(
                    x5cat[32:43, 2 * g * NF : 2 * (g + 1) * NF], ps, AF.Sigmoid,
                    bias=b52t[:],
                )

            mlp_layer(w51, 8, 1, 20, h4, 0, b51t, ev_51, prefetched={0: w51c})
            mlp_layer(w52, 8, 1, 11, h4, 8, b52t, ev_52, prefetched={0: w52c})

            # wv broadcast to all partitions (PE hits this after L4/L5)
            pwv = psum.tile([P, 2 * NF], F32, tag="mm", name="pwv")
            nc.tensor.matmul(pwv[:, :32], onesp[:], wvp[:], start=True, stop=True)
            wv20 = const.tile([P, 20], F32, tag="wv20")
            nc.vector.tensor_copy(wv20[:], pwv[:, :20])

            # ---- transpose the combined head tile: 16 chunks of [32,128] ----
            x5t = tp.tile([P, CH, 43], F32, tag="x5t")
            for c in range(CH):
                pt = psum.tile([P, 2 * NF], F32, tag="mm", name="pt")
                nc.tensor.transpose(
                    pt[:, :64], x5cat[:, c * P : (c + 1) * P], ident[:64, :64]
                )
                if c < 8 and c % 2 == 0:
                    nc.vector.tensor_copy(x5t[:, c, :], pt[:, :43])
                else:
                    nc.scalar.copy(x5t[:, c, :], pt[:, :43])

            # ---- tail part 2: per-head analytic QP ----
            # Two chunk-halves: the first half's vector math overlaps the
            # scalar-engine transpose copies of the second half.
            g12 = tp.tile([P, CH, 2], F32, tag="g12", name="g12")
            nc.vector.tensor_copy(g12[:, :, 0:1], g1[:])
            nc.vector.tensor_copy(g12[:, :, 1:2], g2[:])
            apb = t3("apb", 10)
            ab = t3("ab", 10)
            hv = t3("hv", 10)
            hv2 = t3("hv2", 10)
            refg = tp.tile([P, CH, 10, 2], F32, tag="refg", name="refg")
            viol = t3("viol", 10)
            lam = t3("lam", 10)
            wlam = t3("wlam", 10)
            S = t3("S")
            wref = t3("wref", 20)
            rbxy = t3("rbxy", 2)
            rtxy = t3("rtxy", 2)
            ot = t3("ot", 2)

            for c0, c1 in ((0, CH // 2), (CH // 2, CH)):
                W = c1 - c0
                BW10 = [P, W, 10]
                cs_ = (slice(None), slice(c0, c1))
                alpha = x5t[:, c0:c1, 32:33]
                betas = x5t[:, c0:c1, 33:43]
                nc.vector.tensor_add(apb[*cs_], betas, alpha.to_broadcast(BW10))
                nc.vector.tensor_mul(ab[*cs_], betas, alpha.to_broadcast(BW10))
                nc.vector.tensor_mul(
                    hv[*cs_], apb[*cs_], b_dot[:, c0:c1, :].to_broadcast(BW10)
                )
                nc.vector.tensor_mul(
                    hv2[*cs_], ab[*cs_], barrier[:, c0:c1, :].to_broadcast(BW10)
                )
                nc.vector.tensor_add(hv[*cs_], hv[*cs_], hv2[*cs_])
                nc.vector.tensor_add(
                    hv[*cs_], hv[*cs_], lf2b[:, c0:c1, :].to_broadcast(BW10)
                )
                nc.vector.tensor_mul(
                    refg[:, c0:c1],
                    x5t[:, c0:c1, 0:20].rearrange("p c (h two) -> p c h two", two=2),
                    g12[:, c0:c1, None, :].to_broadcast([P, W, 10, 2]),
                )
                nc.vector.reduce_sum(viol[*cs_, slice(None)], refg[:, c0:c1], axis=AX.X)
                nc.vector.tensor_sub(viol[*cs_], viol[*cs_], hv[*cs_])
                nc.vector.tensor_mul(
                    lam[*cs_], viol[*cs_], igdot[:, c0:c1, :].to_broadcast(BW10)
                )
                nc.vector.tensor_scalar_max(lam[*cs_], lam[*cs_], 0.0)
                nc.vector.tensor_mul(
                    wlam[*cs_], lam[*cs_], wv20[:, None, 0::2].to_broadcast(BW10)
                )
                nc.vector.reduce_sum(S[*cs_, 0], wlam[*cs_], axis=AX.X)
                nc.vector.tensor_mul(
                    wref[*cs_], x5t[:, c0:c1, 0:20],
                    wv20[:, None, :].to_broadcast([P, W, 20]),
                )
                nc.vector.reduce_sum(
                    rbxy[*cs_, slice(None)],
                    wref[*cs_].rearrange("p c (h two) -> p c two h", two=2),
                    axis=AX.X,
                )
                nc.vector.tensor_mul(
                    rtxy[*cs_], g12[:, c0:c1], S[:, c0:c1, :].to_broadcast([P, W, 2])
                )
                nc.vector.tensor_sub(rtxy[*cs_], rtxy[*cs_], rbxy[*cs_])
                nc.vector.tensor_scalar(
                    ot[*cs_, 0], rtxy[*cs_, 0], mlt[:, 0:1], islt[:, 0:1],
                    op0=ALU.subtract, op1=ALU.mult,
                )
                nc.vector.tensor_scalar(
                    ot[*cs_, 1], rtxy[*cs_, 1], mlt[:, 1:2], islt[:, 1:2],
                    op0=ALU.subtract, op1=ALU.mult,
                )

            nc.sync.dma_start(out, ot[:])

    nc.compile()
    return nc


def configure(fp8_layers):
    """Select fp8 layers; must be called before the first kernel() call."""
    global FP8_LAYERS, _CACHED_NC
    if frozenset(fp8_layers) != FP8_LAYERS:
        FP8_LAYERS = frozenset(fp8_layers)
        _CACHED_NC = None


def _get_nc():
    global _CACHED_NC
    if _CACHED_NC is None:
        _CACHED_NC = _build(FP8_LAYERS)
    return _CACHED_NC


def _bf16(a):
    return np.ascontiguousarray(a.astype(ml_dtypes.bfloat16))


def _f32(a):
    return np.ascontiguousarray(np.asarray(a, dtype=np.float32))


def _e4(a):
    dt = mybir.dt.np(mybir.dt.float8e4)
    return np.ascontiguousarray(a.astype(dt))


def _prep_inputs(inputs):
    x = _f32(inputs["x"])
    mean = _f32(inputs["mean"])
    std = _f32(inputs["std"])
    mean_label = _f32(inputs["mean_label"])
    std_label = _f32(inputs["std_label"])
    wt = _f32(inputs["wt"])
    W1, b1 = _f32(inputs["W1"]), _f32(inputs["b1"])
    W2, b2 = _f32(inputs["W2"]), _f32(inputs["b2"])
    W31, b31 = _f32(inputs["W31"]), _f32(inputs["b31"])
    W32, b32 = _f32(inputs["W32"]), _f32(inputs["b32"])
    W41, b41 = _f32(inputs["W41"]), _f32(inputs["b41"])
    W42, b42 = _f32(inputs["W42"]), _f32(inputs["b42"])
    W51, b51 = _f32(inputs["W51"]), _f32(inputs["b51"])
    W52, b52 = _f32(inputs["W52"]), _f32(inputs["b52"])

    def pack4(wT, KT, MT):  # (K, M) -> (128, MT, KT, 128)
        return _bf16(wT.reshape(KT, P, MT, P).transpose(1, 2, 0, 3))

    def pack3(wT, KT):  # (K, M) -> (128, KT, M)
        K, M = wT.shape
        return _bf16(wT.reshape(KT, P, M).transpose(1, 0, 2))

    W3T = np.concatenate([W31.T, W32.T], axis=1)  # (2048, 2048)
    b3 = np.concatenate([b31, b32])

    # fp8 calibration: static activation scales from a row subsample
    # (4.7x headroom to the e4m3 max), per-tensor weight scales.
    sa1 = sa2 = sa3 = 1.0
    sw2 = sw3 = sw4 = 1.0
    if FP8_LAYERS:
        xs = x[:512]
        h1s = np.maximum(xs @ W1.T + b1, 0.0)
        if "l2" in FP8_LAYERS:
            sa1 = 96.0 / max(float(np.abs(h1s).max()), 1e-30)
            sw2 = 192.0 / max(float(np.abs(W2).max()), 1e-30)
        if "l3" in FP8_LAYERS or "l4" in FP8_LAYERS:
            h2s = np.maximum(h1s @ W2.T + b2, 0.0)
            if "l3" in FP8_LAYERS:
                sa2 = 96.0 / max(float(np.abs(h2s).max()), 1e-30)
                sw3 = 192.0 / max(float(np.abs(W3T).max()), 1e-30)
            if "l4" in FP8_LAYERS:
                h3s = np.maximum(h2s @ W3T + b3, 0.0)
                sa3 = 96.0 / max(float(np.abs(h3s).max()), 1e-30)
                sw4 = 192.0 / max(
                    float(max(np.abs(W41).max(), np.abs(W42).max())), 1e-30
                )
    esc2 = sa2 / (sw2 * sa1)
    esc3 = sa3 / (sw3 * sa2)
    esc4 = 1.0 / (sw4 * sa3)

    def wpack(wT, KT, MT, sw, fp8):
        packed = wT.reshape(KT, P, MT, P).transpose(1, 2, 0, 3)
        if fp8:
            return _e4(packed * sw)
        return _bf16(packed)

    w1p = np.zeros((P, 2048), np.float32)
    w1p[:4] = W1.T
    shared = {
        "w1": _bf16(w1p * sa1),
        "w2": wpack(W2.T, 16, 16, sw2, "l2" in FP8_LAYERS),
        "w3": wpack(W3T, 16, 16, sw3, "l3" in FP8_LAYERS),
        "w41": wpack(W41.T, 8, 8, sw4, "l4" in FP8_LAYERS),
        "w42": wpack(W42.T, 8, 8, sw4, "l4" in FP8_LAYERS),
        "w51": pack3(W51.T, 8),
        "w52": pack3(W52.T, 8),
        "esc2": _f32(np.full((P, 1), esc2)),
        "esc3": _f32(np.full((P, 1), esc3)),
        "esc4": _f32(np.full((P, 1), esc4)),
        "b1": _f32(b1.reshape(16, P).T * sa1),
        "b2": _f32(b2.reshape(16, P).T * sa2),
        "b3": _f32(b3.reshape(16, P).T * sa3),
        "b41": _f32(b41.reshape(8, P).T),
        "b42": _f32(b42.reshape(8, P).T),
        "b51": b51,
        "b52": b52,
        "stdb": _f32(np.tile(std[None, :], (P, 1))),
        "meanb": _f32(np.tile(mean[None, :], (P, 1))),
        "mlb": _f32(np.tile(mean_label[None, :], (P, 1))),
        "islb": _f32(np.tile((1.0 / std_label)[None, :], (P, 1))),
        "wtv": wt,
    }

    in_maps = []
    for i in range(N_CORES):
        xs = x[i * B : (i + 1) * B]  # (2048, 4)
        m = dict(shared)
        xtp = np.zeros((P, B), np.float32)
        xtp[:4] = xs.T
        m["xt"] = _bf16(xtp)
        m["xn"] = _f32(xs.reshape(CH, P, 4).transpose(1, 0, 2))
        in_maps.append(m)
    return in_maps


def kernel_core(inputs, trace=False):
    nc = _get_nc()
    in_maps = _prep_inputs(inputs)
    res = run_bass_kernel_spmd(
        nc, in_maps, core_ids=list(range(N_CORES)), trace=trace
    )
    shards = []
    for i in range(N_CORES):
        o = res.results[i]["out"]  # (128, 16, 2)
        shards.append(o.transpose(1, 0, 2).reshape(B, 2))
    return np.concatenate(shards, axis=0).astype(np.float32), res


def kernel(**inputs):
    out, _ = kernel_core(inputs)
    return out


# revision 27
# speedup vs baseline: 2.1983x; 1.0009x over previous
"""Trainium2 Bass kernel for nn_ABNet_U (multi-branch MLP + CBF-QP head).

Data-parallel over batch: 16384 rows -> 8 NeuronCores x 2048 rows, weights
replicated and host-prepped into K-major layouts.  The three large middle
GEMMs (L2/L3/L4) run in fp8-e4m3 with DoubleRow perf mode (2 weights per
PE cell -> half the matmul count); L1 and the small heads stay bf16.  All
GEMMs accumulate in fp32 PSUM with fused scale+bias+activation eviction on
the Scalar/Vector engines (fp8 rescales fold into the eviction scale).
The trig/barrier/QP tail runs in fp32 on the Vector engine with batch on
partitions, emitted early so it executes underneath the GEMM phase.
Measured: rel err 1.40e-2 vs the fp32 reference (gate 2e-2), ~336us/core
at full PE clock.  Set FP8_LAYERS = frozenset() for the bf16-exact
fallback (rel err 9.3e-4, ~612us).
"""

import sys

sys.path.insert(0, "/opt/trn_rl_repo")

import numpy as np
import ml_dtypes

import concourse.bass as bass
import concourse.mybir as mybir
import concourse.tile as tile
from concourse import bacc
from concourse.bass_utils import run_bass_kernel_spmd
from concourse.masks import make_identity

N_CORES = 8
B_GLOBAL = 16384
B = B_GLOBAL // N_CORES  # 2048 rows per core
P = 128
CH = B // P              # 16 batch chunks of 128 (tail layout)
NF = 512                 # matmul free-dim chunk
NB = B // NF             # 4 free chunks
HEADS = 10

AF = mybir.ActivationFunctionType
ALU = mybir.AluOpType
AX = mybir.AxisListType
F32 = mybir.dt.float32
BF16 = mybir.dt.bfloat16
I32 = mybir.dt.int32

TWO_PI = float(2.0 * np.pi)
HALF_PI = float(0.5 * np.pi)

# Layers computed in fp8-e4m3 with DoubleRow (2 weights/PE cell, halves the
# matmul count).  Weights use per-tensor scales; activations use static
# per-layer scales calibrated host-side on a row subsample with 4.7x
# headroom to the e4m3 max.
FP8_LAYERS = frozenset({"l2", "l3", "l4"})  # override via configure()
FP8 = mybir.dt.float8e4

_CACHED_NC = None


def _build(fp8_layers=frozenset()):
    nc = bacc.Bacc(
        "TRN2",
        target_bir_lowering=False,
        debug=False,
        enable_asserts=False,
        num_devices=N_CORES,
    )

    def din(name, shape, dt=F32):
        return nc.dram_tensor(name, list(shape), dt, kind="ExternalInput").ap()

    xt = din("xt", (P, B), BF16)
    xn = din("xn", (P, CH, 4))              # x shard, [p, chunk, feat] fp32
    w1 = din("w1", (P, 2048), BF16)
    w2 = din("w2", (P, 16, 16, P), FP8 if "l2" in fp8_layers else BF16)
    w3 = din("w3", (P, 16, 16, P), FP8 if "l3" in fp8_layers else BF16)
    w41 = din("w41", (P, 8, 8, P), FP8 if "l4" in fp8_layers else BF16)
    w42 = din("w42", (P, 8, 8, P), FP8 if "l4" in fp8_layers else BF16)
    esc2 = din("esc2", (P, 1))
    esc3 = din("esc3", (P, 1))
    esc4 = din("esc4", (P, 1))
    w51 = din("w51", (P, 8, 20), BF16)      # [p, kt, m]
    w52 = din("w52", (P, 8, 11), BF16)
    b1 = din("b1", (P, 16))
    b2 = din("b2", (P, 16))
    b3 = din("b3", (P, 16))
    b41 = din("b41", (P, 8))
    b42 = din("b42", (P, 8))
    b51 = din("b51", (20,))
    b52 = din("b52", (11,))
    stdb = din("stdb", (P, 4))
    meanb = din("meanb", (P, 4))
    mlb = din("mlb", (P, 2))
    islb = din("islb", (P, 2))
    wtv = din("wtv", (10,))
    out = nc.dram_tensor("out", [P, CH, 2], F32, kind="ExternalOutput").ap()

    with tile.TileContext(nc) as tc:
        from contextlib import ExitStack

        with ExitStack() as ctx:
            const = ctx.enter_context(tc.tile_pool(name="const", bufs=1))
            wpool = ctx.enter_context(tc.tile_pool(name="wpool", bufs=3))
            hpool = ctx.enter_context(tc.tile_pool(name="hpool", bufs=2))
            psum = ctx.enter_context(tc.tile_pool(name="psum", bufs=4, space="PSUM"))
            tp = ctx.enter_context(tc.tile_pool(name="tp", bufs=1))

            # ---- L1-critical loads first: keep the PE fed from t=0 ----
            b1t = const.tile([P, 16], F32, tag="b1")
            nc.sync.dma_start(b1t[:], b1)
            xtb = const.tile([P, B], BF16, tag="xtb")
            nc.sync.dma_start(xtb[:], xt)
            w1tb = const.tile([P, 2048], BF16, tag="w1tb")
            nc.sync.dma_start(w1tb[:], w1)

            # prefetch: L2's first weight column gates the L1->L2 handoff;
            # the tiny L5 weight tiles ride the otherwise-idle gpsimd queue.
            wcol2_0 = wpool.tile(
                [P, 16, P], FP8 if "l2" in fp8_layers else BF16,
                tag="wcol", name="wcol2_0",
            )
            nc.sync.dma_start(wcol2_0[:], w2[:, 0])
            w51c = const.tile([P, 8, 20], BF16, tag="w51c")
            nc.gpsimd.dma_start(w51c[:], w51)
            w52c = const.tile([P, 8, 11], BF16, tag="w52c")
            nc.gpsimd.dma_start(w52c[:], w52)

            # PE warm-up: dummy matmuls fill the idle window while the L1
            # input DMAs land, and push the HAM clock gate to 8/8 before the
            # real matmul stream begins.
            wrm = const.tile([P, NF], BF16, tag="wrm")
            nc.vector.memset(wrm[:], 0.0)
            wps = psum.tile([P, 2 * NF], F32, tag="mm", name="wps")
            for _ in range(16):
                nc.tensor.matmul(
                    wps[:, :NF], wrm[:, :P], wrm[:], start=True, stop=True
                )

            # warm the ACT engine's function tables in the startup window so
            # the 1.3us table loads don't land mid-stream later
            tbl = const.tile([1, 2], F32, tag="tbl")
            nc.vector.memset(tbl[:], 0.25)
            nc.scalar.activation(tbl[:, 1:2], tbl[:, 0:1], AF.Relu)
            nc.scalar.activation(tbl[:, 1:2], tbl[:, 0:1], AF.Exp)
            nc.scalar.activation(tbl[:, 1:2], tbl[:, 0:1], AF.Sin)
            nc.scalar.activation(tbl[:, 1:2], tbl[:, 0:1], AF.Sigmoid)
            nc.scalar.activation(tbl[:, 1:2], tbl[:, 0:1], AF.Identity)

            # ---- L1: h1 = relu(W1 @ x^T + b1), K=4 zero-padded to 128 ----
            # One matmul per eviction, so evictions bound this layer: split
            # them across the Scalar and Vector engines.
            h1dt = FP8 if "l2" in fp8_layers else BF16
            h2dt = FP8 if "l3" in fp8_layers else BF16
            h3dt = FP8 if "l4" in fp8_layers else BF16
            h1 = hpool.tile([P, 16, B], h1dt, tag="act", name="h1")
            flip = 0
            for g in range(NB // 2):
                for m in range(16):
                    ps = psum.tile([P, 2 * NF], F32, tag="mm")
                    for half in range(2):
                        n = 2 * g + half
                        nc.tensor.matmul(
                            ps[:, half * NF : (half + 1) * NF],
                            w1tb[:, m * P : (m + 1) * P],
                            xtb[:, n * NF : (n + 1) * NF],
                            start=True,
                            stop=True,
                        )
                    dst = h1[:, m, 2 * g * NF : 2 * (g + 1) * NF]
                    # ACT evicts ~1.45us vs DVE ~1.78us: 9/16 duty on ACT
                    if flip % 16 in (0, 2, 4, 6, 8, 10, 12, 14, 15):
                        nc.scalar.activation(
                            dst, ps[:], AF.Relu, bias=b1t[:, m : m + 1]
                        )
                    else:
                        nc.vector.tensor_scalar(
                            dst, ps[:], b1t[:, m : m + 1], 0.0,
                            op0=ALU.add, op1=ALU.max,
                        )
                    flip += 1
                    # (sa1 activation scale is folded into W1/b1 on the host)

            # ---- remaining constants (emitted after L1 so they never gate it)
            b2t = const.tile([P, 16], F32, tag="b2")
            nc.sync.dma_start(b2t[:], b2)
            b3t = const.tile([P, 16], F32, tag="b3")
            nc.sync.dma_start(b3t[:], b3)
            b41t = const.tile([P, 8], F32, tag="b41")
            nc.sync.dma_start(b41t[:], b41)
            b42t = const.tile([P, 8], F32, tag="b42")
            nc.sync.dma_start(b42t[:], b42)
            b51t = const.tile([20, 1], F32, tag="b51")
            nc.sync.dma_start(b51t[:], b51[:, None])
            b52t = const.tile([11, 1], F32, tag="b52")
            nc.sync.dma_start(b52t[:], b52[:, None])
            stdt = const.tile([P, 4], F32, tag="stdt")
            nc.sync.dma_start(stdt[:], stdb)
            meant = const.tile([P, 4], F32, tag="meant")
            nc.sync.dma_start(meant[:], meanb)
            mlt = const.tile([P, 2], F32, tag="mlt")
            nc.sync.dma_start(mlt[:], mlb)
            islt = const.tile([P, 2], F32, tag="islt")
            nc.sync.dma_start(islt[:], islb)
            esc2t = const.tile([P, 1], F32, tag="esc2t")
            nc.sync.dma_start(esc2t[:], esc2)
            esc3t = const.tile([P, 1], F32, tag="esc3t")
            nc.sync.dma_start(esc3t[:], esc3)
            esc4t = const.tile([P, 1], F32, tag="esc4t")
            nc.sync.dma_start(esc4t[:], esc4)
            halfpi = const.tile([P, 1], F32, tag="halfpi")
            nc.vector.memset(halfpi[:], HALF_PI)
            ident = const.tile([P, P], F32)
            make_identity(nc, ident[:])

            # softmax(wt) DVE chain (PE broadcast deferred until after L4)
            wtt = const.tile([1, 10], F32, tag="wtt")
            nc.sync.dma_start(wtt[:], wtv[None, :])
            mx = const.tile([1, 1], F32, tag="mx")
            nc.vector.reduce_max(mx[:, 0:1], wtt[:], axis=AX.X)
            nm = const.tile([1, 1], F32, tag="nm")
            nc.vector.tensor_scalar_mul(nm[:], mx[:], -1.0)
            ex = const.tile([1, 10], F32, tag="ex")
            nc.scalar.activation(ex[:], wtt[:], AF.Exp, bias=nm[:])
            sm = const.tile([1, 1], F32, tag="sm")
            nc.vector.reduce_sum(sm[:, 0:1], ex[:], axis=AX.X)
            inv = const.tile([1, 1], F32, tag="inv")
            nc.vector.reciprocal(inv[:], sm[:])
            wv10 = const.tile([1, 10], F32, tag="wv10")
            nc.vector.tensor_scalar_mul(wv10[:], ex[:], inv[:])
            wvp = const.tile([32, 32], F32, tag="wvp")
            nc.vector.memset(wvp[:], 0.0)
            nc.vector.tensor_copy(
                wvp[0:1, 0:20].rearrange("p (h c) -> p h c", c=2),
                wv10[:, :, None].to_broadcast([1, 10, 2]),
            )
            onesp = const.tile([32, P], F32, tag="onesp")
            nc.vector.memset(onesp[:], 0.0)
            nc.vector.memset(onesp[0:1, :], 1.0)

            # combined x51/x52 head tile (rows 0..19 = x51, 20..30 = x52)
            x5cat = tp.tile([64, B], F32, tag="x5cat")
            nc.vector.memset(x5cat[:], 0.0)

            # ---- tail part 1: geometry from x only — emitted now so the
            # Vector engine computes it underneath the L2..L5 GEMMs.
            def t3(tag, d=1):
                return tp.tile([P, CH, d], F32, tag=tag, name=tag)

            xnt = t3("xnt", 4)
            nc.sync.dma_start(xnt[:], xn)
            x0 = t3("x0", 4)
            nc.vector.tensor_mul(
                x0[:], xnt[:], stdt[:, None, :].to_broadcast([P, CH, 4])
            )
            nc.vector.tensor_add(
                x0[:], x0[:], meant[:, None, :].to_broadcast([P, CH, 4])
            )

            th = x0[:, :, 0::2]   # [P, CH, 2] angles
            wv_ = x0[:, :, 1::2]  # [P, CH, 2] angular velocities

            # range-reduce th -> rs in [-pi, pi]:  rs = th - 2pi*rint(th/2pi)
            q = t3("q", 2)
            qi = tp.tile([P, CH, 2], I32, tag="qi")
            qr = t3("qr", 2)
            rs = t3("rs", 2)
            nc.vector.tensor_scalar_mul(q[:], th, 1.0 / TWO_PI)
            nc.vector.tensor_copy(qi[:], q[:])
            nc.vector.tensor_copy(qr[:], qi[:])
            nc.vector.scalar_tensor_tensor(
                rs[:], in0=qr[:], scalar=-TWO_PI, in1=th,
                op0=ALU.mult, op1=ALU.add,
            )
            # range-reduce th + pi/2 -> rc (for cos)
            qc = t3("qc", 2)
            qci = tp.tile([P, CH, 2], I32, tag="qci")
            qcr = t3("qcr", 2)
            rc = t3("rc", 2)
            nc.vector.tensor_scalar(
                qc[:], th, 1.0 / TWO_PI, 0.25, op0=ALU.mult, op1=ALU.add
            )
            nc.vector.tensor_copy(qci[:], qc[:])
            nc.vector.tensor_copy(qcr[:], qci[:])
            nc.vector.scalar_tensor_tensor(
                rc[:], in0=qcr[:], scalar=-TWO_PI, in1=th,
                op0=ALU.mult, op1=ALU.add,
            )
            nc.vector.tensor_scalar_add(rc[:], rc[:], HALF_PI)

            sn = t3("sn", 2)
            cs = t3("cs", 2)
            nc.scalar.activation(sn[:], rs[:], AF.Sin)
            nc.scalar.activation(cs[:], rc[:], AF.Sin)

            s1, s2 = sn[:, :, 0:1], sn[:, :, 1:2]
            c1, c2 = cs[:, :, 0:1], cs[:, :, 1:2]
            w1v, w2v = wv_[:, :, 0:1], wv_[:, :, 1:2]

            px = t3("px")
            nc.vector.tensor_add(px[:], c1, c2)
            nc.vector.tensor_scalar_mul(px[:], px[:], 3.0)
            py = t3("py")
            nc.vector.tensor_add(py[:], s1, s2)
            nc.vector.tensor_scalar(py[:], py[:], 3.0, -7.0, op0=ALU.mult, op1=ALU.add)

            s1w = t3("s1w")
            nc.vector.tensor_mul(s1w[:], s1, w1v)
            s2w = t3("s2w")
            nc.vector.tensor_mul(s2w[:], s2, w2v)
            vx = t3("vx")
            nc.vector.tensor_add(vx[:], s1w[:], s2w[:])
            nc.vector.tensor_scalar_mul(vx[:], vx[:], -3.0)
            c1w = t3("c1w")
            nc.vector.tensor_mul(c1w[:], c1, w1v)
            c2w = t3("c2w")
            nc.vector.tensor_mul(c2w[:], c2, w2v)
            vy = t3("vy")
            nc.vector.tensor_add(vy[:], c1w[:], c2w[:])
            nc.vector.tensor_scalar_mul(vy[:], vy[:], 3.0)

            pxx = t3("pxx")
            nc.vector.tensor_mul(pxx[:], px[:], px[:])
            pyy = t3("pyy")
            nc.vector.tensor_mul(pyy[:], py[:], py[:])
            # barrier scaled by 16 = alpha*beta scale (4*sigmoid each)
            barrier = t3("barrier")
            nc.vector.tensor_add(barrier[:], pxx[:], pyy[:])
            nc.vector.tensor_scalar(
                barrier[:], barrier[:], 16.0, -256.0, op0=ALU.mult, op1=ALU.add
            )

            pv1 = t3("pv1")
            nc.vector.tensor_mul(pv1[:], px[:], vx[:])
            pv2 = t3("pv2")
            nc.vector.tensor_mul(pv2[:], py[:], vy[:])
            b_dot = t3("b_dot")
            nc.vector.tensor_add(b_dot[:], pv1[:], pv2[:])
            # 2 (from b_dot) * 4 (alpha+beta sigmoid scale)
            nc.vector.tensor_scalar_mul(b_dot[:], b_dot[:], 8.0)

            w1sq = t3("w1sq")
            nc.vector.tensor_mul(w1sq[:], w1v, w1v)
            w2sq = t3("w2sq")
            nc.vector.tensor_mul(w2sq[:], w2v, w2v)
            ca = t3("ca")
            nc.vector.tensor_mul(ca[:], c1, w1sq[:])
            cb = t3("cb")
            nc.vector.tensor_mul(cb[:], c2, w2sq[:])
            nc.vector.tensor_add(ca[:], ca[:], cb[:])   # c1*w1^2 + c2*w2^2
            sa = t3("sa")
            nc.vector.tensor_mul(sa[:], s1, w1sq[:])
            sb = t3("sb")
            nc.vector.tensor_mul(sb[:], s2, w2sq[:])
            nc.vector.tensor_add(sa[:], sa[:], sb[:])   # s1*w1^2 + s2*w2^2

            vxx = t3("vxx")
            nc.vector.tensor_mul(vxx[:], vx[:], vx[:])
            vyy = t3("vyy")
            nc.vector.tensor_mul(vyy[:], vy[:], vy[:])
            vsum = t3("vsum")
            nc.vector.tensor_add(vsum[:], vxx[:], vyy[:])
            nc.vector.tensor_scalar_mul(vsum[:], vsum[:], 2.0)  # 2vx^2+2vy^2

            pca = t3("pca")
            nc.vector.tensor_mul(pca[:], px[:], ca[:])
            psa = t3("psa")
            nc.vector.tensor_mul(psa[:], py[:], sa[:])
            nc.vector.tensor_add(pca[:], pca[:], psa[:])
            lf2b = t3("lf2b")
            nc.vector.scalar_tensor_tensor(
                lf2b[:], in0=pca[:], scalar=-6.0, in1=vsum[:],
                op0=ALU.mult, op1=ALU.add,
            )  # Lf2b = 2(vx^2+vy^2) - 6*(px*ca + py*sa)

            g1 = t3("g1")
            m1 = t3("m1")
            nc.vector.tensor_mul(m1[:], py[:], c1)
            m2 = t3("m2")
            nc.vector.tensor_mul(m2[:], px[:], s1)
            nc.vector.tensor_sub(g1[:], m1[:], m2[:])
            nc.vector.tensor_scalar_mul(g1[:], g1[:], 6.0)
            g2 = t3("g2")
            nc.vector.tensor_mul(m1[:], py[:], c2)
            nc.vector.tensor_mul(m2[:], px[:], s2)
            nc.vector.tensor_sub(g2[:], m1[:], m2[:])
            nc.vector.tensor_scalar_mul(g2[:], g2[:], 6.0)

            gdot = t3("gdot")
            g1sq = t3("g1sq")
            nc.vector.tensor_mul(g1sq[:], g1[:], g1[:])
            g2sq = t3("g2sq")
            nc.vector.tensor_mul(g2sq[:], g2[:], g2[:])
            nc.vector.tensor_add(gdot[:], g1sq[:], g2sq[:])
            igdot = t3("igdot")
            nc.vector.reciprocal(igdot[:], gdot[:])

            # ---- generic streamed GEMM layer ----
            def mlp_layer(wdram, KT, MT, MD, hin, kin_base, btile, evict,
                          prefetched=None, dr=False):
                wdt = FP8 if dr else BF16
                for m in range(MT):
                    mp = min(P, MD - m * P)
                    if prefetched is not None and m in prefetched:
                        wcol = prefetched[m]
                    else:
                        wcol = wpool.tile([P, KT, mp], wdt, tag="wcol")
                        if len(wdram.shape) == 4:
                            nc.sync.dma_start(wcol[:], wdram[:, m])
                        else:
                            nc.sync.dma_start(wcol[:], wdram)
                    for g in range(NB // 2):
                        ps = psum.tile([P, 2 * NF], F32, tag="mm")
                        for half in range(2):
                            n = 2 * g + half
                            if dr:
                                for k2 in range(KT // 2):
                                    nc.tensor.matmul(
                                        ps[:mp, half * NF : (half + 1) * NF],
                                        wcol[:, 2 * k2 : 2 * k2 + 2, :],
                                        hin[:, kin_base + 2 * k2 : kin_base + 2 * k2 + 2,
                                            n * NF : (n + 1) * NF],
                                        start=(k2 == 0),
                                        stop=(k2 == KT // 2 - 1),
                                        perf_mode=mybir.MatmulPerfMode.DoubleRow,
                                    )
                            else:
                                for k in range(KT):
                                    nc.tensor.matmul(
                                        ps[:mp, half * NF : (half + 1) * NF],
                                        wcol[:, k, :],
                                        hin[:, kin_base + k, n * NF : (n + 1) * NF],
                                        start=(k == 0),
                                        stop=(k == KT - 1),
                                    )
                        evict(m, g, ps[:mp])

            # ---- L2 / L3 / L4 ----
            h2 = hpool.tile([P, 16, B], h2dt, tag="act", name="h2")

            def ev_h(hout, btile, m_off=0, scale=1.0):
                def _e(m, g, ps):
                    nc.scalar.activation(
                        hout[:, m_off + m, 2 * g * NF : 2 * (g + 1) * NF], ps,
                        AF.Relu, bias=btile[:, m : m + 1], scale=scale,
                    )
                return _e

            sc2 = esc2t[:]
            mlp_layer(w2, 16, 16, 2048, h1, 0, b2t, ev_h(h2, b2t, 0, sc2),
                      prefetched={0: wcol2_0}, dr="l2" in fp8_layers)

            h3 = hpool.tile([P, 16, B], h3dt, tag="act", name="h3")
            sc3 = esc3t[:]
            mlp_layer(w3, 16, 16, 2048, h2, 0, b3t, ev_h(h3, b3t, 0, sc3),
                      dr="l3" in fp8_layers)

            h4 = hpool.tile([P, 16, B], BF16, tag="act", name="h4")
            sc4 = esc4t[:]
            mlp_layer(w41, 8, 8, 1024, h3, 0, b41t, ev_h(h4, b41t, 0, sc4),
                      dr="l4" in fp8_layers)
            mlp_layer(w42, 8, 8, 1024, h3, 8, b42t, ev_h(h4, b42t, 8, sc4),
                      dr="l4" in fp8_layers)

            # ---- L5 into the combined head tile ----
            def ev_51(m, g, ps):
                nc.vector.tensor_scalar_add(
                    x5cat[:20, 2 * g * NF : 2 * (g + 1) * NF], ps, b51t[:]
                )

            def ev_52(m, g, ps):
                nc.scalar.activation(
                    x5cat[32:43, 2 * g * NF : 2 * (g + 1) * NF], ps, AF.Sigmoid,
                    bias=b52t[:],
                )

            mlp_layer(w51, 8, 1, 20, h4, 0, b51t, ev_51, prefetched={0: w51c})
            mlp_layer(w52, 8, 1, 11, h4, 8, b52t, ev_52, prefetched={0: w52c})

            # wv broadcast to all partitions (PE hits this after L4/L5)
            pwv = psum.tile([P, 2 * NF], F32, tag="mm", name="pwv")
            nc.tensor.matmul(pwv[:, :32], onesp[:], wvp[:], start=True, stop=True)
            wv20 = const.tile([P, 20], F32, tag="wv20")
            nc.vector.tensor_copy(wv20[:], pwv[:, :20])

            # ---- transpose the combined head tile: 16 chunks of [32,128] ----
            x5t = tp.tile([P, CH, 43], F32, tag="x5t")
            for c in range(CH):
                pt = psum.tile([P, 2 * NF], F32, tag="mm", name="pt")
                nc.tensor.transpose(
                    pt[:, :64], x5cat[:, c * P : (c + 1) * P], ident[:64, :64]
                )
                if c < 8 and c % 2 == 0:
                    nc.vector.tensor_copy(x5t[:, c, :], pt[:, :43])
                else:
                    nc.scalar.copy(x5t[:, c, :], pt[:, :43])

            # ---- tail part 2: per-head analytic QP ----
            # Two chunk-halves: the first half's vector math overlaps the
            # scalar-engine transpose copies of the second half.
            g12 = tp.tile([P, CH, 2], F32, tag="g12", name="g12")
            nc.vector.tensor_copy(g12[:, :, 0:1], g1[:])
            nc.vector.tensor_copy(g12[:, :, 1:2], g2[:])
            apb = t3("apb", 10)
            ab = t3("ab", 10)
            hv = t3("hv", 10)
            hv2 = t3("hv2", 10)
            refg = tp.tile([P, CH, 10, 2], F32, tag="refg", name="refg")
            viol = t3("viol", 10)
            lam = t3("lam", 10)
            wlam = t3("wlam", 10)
            S = t3("S")
            wref = t3("wref", 20)
            rbxy = t3("rbxy", 2)
            rtxy = t3("rtxy", 2)
            ot = t3("ot", 2)

            for c0, c1 in ((0, CH // 2), (CH // 2, CH)):
                W = c1 - c0
                BW10 = [P, W, 10]
                cs_ = (slice(None), slice(c0, c1))
                alpha = x5t[:, c0:c1, 32:33]
                betas = x5t[:, c0:c1, 33:43]
                nc.vector.tensor_add(apb[*cs_], betas, alpha.to_broadcast(BW10))
                nc.vector.tensor_mul(ab[*cs_], betas, alpha.to_broadcast(BW10))
                nc.vector.tensor_mul(
                    hv[*cs_], apb[*cs_], b_dot[:, c0:c1, :].to_broadcast(BW10)
                )
                nc.vector.tensor_mul(
                    hv2[*cs_], ab[*cs_], barrier[:, c0:c1, :].to_broadcast(BW10)
                )
                nc.vector.tensor_add(hv[*cs_], hv[*cs_], hv2[*cs_])
                nc.vector.tensor_add(
                    hv[*cs_], hv[*cs_], lf2b[:, c0:c1, :].to_broadcast(BW10)
                )
                nc.vector.tensor_mul(
                    refg[:, c0:c1],
                    x5t[:, c0:c1, 0:20].rearrange("p c (h two) -> p c h two", two=2),
                    g12[:, c0:c1, None, :].to_broadcast([P, W, 10, 2]),
                )
                nc.vector.reduce_sum(viol[*cs_, slice(None)], refg[:, c0:c1], axis=AX.X)
                nc.vector.tensor_sub(viol[*cs_], viol[*cs_], hv[*cs_])
                nc.vector.tensor_mul(
                    lam[*cs_], viol[*cs_], igdot[:, c0:c1, :].to_broadcast(BW10)
                )
                nc.vector.tensor_scalar_max(lam[*cs_], lam[*cs_], 0.0)
                nc.vector.tensor_mul(
                    wlam[*cs_], lam[*cs_], wv20[:, None, 0::2].to_broadcast(BW10)
                )
                nc.vector.reduce_sum(S[*cs_, 0], wlam[*cs_], axis=AX.X)
                nc.vector.tensor_mul(
                    wref[*cs_], x5t[:, c0:c1, 0:20],
                    wv20[:, None, :].to_broadcast([P, W, 20]),
                )
                nc.vector.reduce_sum(
                    rbxy[*cs_, slice(None)],
                    wref[*cs_].rearrange("p c (h two) -> p c two h", two=2),
                    axis=AX.X,
                )
                nc.vector.tensor_mul(
                    rtxy[*cs_], g12[:, c0:c1], S[:, c0:c1, :].to_broadcast([P, W, 2])
                )
                nc.vector.tensor_sub(rtxy[*cs_], rtxy[*cs_], rbxy[*cs_])
                nc.vector.tensor_scalar(
                    ot[*cs_, 0], rtxy[*cs_, 0], mlt[:, 0:1], islt[:, 0:1],
                    op0=ALU.subtract, op1=ALU.mult,
                )
                nc.vector.tensor_scalar(
                    ot[*cs_, 1], rtxy[*cs_, 1], mlt[:, 1:2], islt[:, 1:2],
                    op0=ALU.subtract, op1=ALU.mult,
                )

            nc.sync.dma_start(out, ot[:])

    nc.compile()
    return nc


def configure(fp8_layers):
    """Select fp8 layers; must be called before the first kernel() call."""
    global FP8_LAYERS, _CACHED_NC
    if frozenset(fp8_layers) != FP8_LAYERS:
        FP8_LAYERS = frozenset(fp8_layers)
        _CACHED_NC = None


def _get_nc():
    global _CACHED_NC
    if _CACHED_NC is None:
        _CACHED_NC = _build(FP8_LAYERS)
    return _CACHED_NC


def _bf16(a):
    return np.ascontiguousarray(a.astype(ml_dtypes.bfloat16))


def _f32(a):
    return np.ascontiguousarray(np.asarray(a, dtype=np.float32))


def _e4(a):
    dt = mybir.dt.np(mybir.dt.float8e4)
    return np.ascontiguousarray(a.astype(dt))


def _prep_inputs(inputs):
    x = _f32(inputs["x"])
    mean = _f32(inputs["mean"])
    std = _f32(inputs["std"])
    mean_label = _f32(inputs["mean_label"])
    std_label = _f32(inputs["std_label"])
    wt = _f32(inputs["wt"])
    W1, b1 = _f32(inputs["W1"]), _f32(inputs["b1"])
    W2, b2 = _f32(inputs["W2"]), _f32(inputs["b2"])
    W31, b31 = _f32(inputs["W31"]), _f32(inputs["b31"])
    W32, b32 = _f32(inputs["W32"]), _f32(inputs["b32"])
    W41, b41 = _f32(inputs["W41"]), _f32(inputs["b41"])
    W42, b42 = _f32(inputs["W42"]), _f32(inputs["b42"])
    W51, b51 = _f32(inputs["W51"]), _f32(inputs["b51"])
    W52, b52 = _f32(inputs["W52"]), _f32(inputs["b52"])

    def pack4(wT, KT, MT):  # (K, M) -> (128, MT, KT, 128)
        return _bf16(wT.reshape(KT, P, MT, P).transpose(1, 2, 0, 3))

    def pack3(wT, KT):  # (K, M) -> (128, KT, M)
        K, M = wT.shape
        return _bf16(wT.reshape(KT, P, M).transpose(1, 0, 2))

    W3T = np.concatenate([W31.T, W32.T], axis=1)  # (2048, 2048)
    b3 = np.concatenate([b31, b32])

    # fp8 calibration: static activation scales from a row subsample
    # (4.7x headroom to the e4m3 max), per-tensor weight scales.
    sa1 = sa2 = sa3 = 1.0
    sw2 = sw3 = sw4 = 1.0
    if FP8_LAYERS:
        xs = x[:512]
        h1s = np.maximum(xs @ W1.T + b1, 0.0)
        if "l2" in FP8_LAYERS:
            sa1 = 96.0 / max(float(np.abs(h1s).max()), 1e-30)
            sw2 = 192.0 / max(float(np.abs(W2).max()), 1e-30)
        if "l3" in FP8_LAYERS or "l4" in FP8_LAYERS:
            h2s = np.maximum(h1s @ W2.T + b2, 0.0)
            if "l3" in FP8_LAYERS:
                sa2 = 96.0 / max(float(np.abs(h2s).max()), 1e-30)
                sw3 = 192.0 / max(float(np.abs(W3T).max()), 1e-30)
            if "l4" in FP8_LAYERS:
                h3s = np.maximum(h2s @ W3T + b3, 0.0)
                sa3 = 96.0 / max(float(np.abs(h3s).max()), 1e-30)
                sw4 = 192.0 / max(
                    float(max(np.abs(W41).max(), np.abs(W42).max())), 1e-30
                )
    esc2 = sa2 / (sw2 * sa1)
    esc3 = sa3 / (sw3 * sa2)
    esc4 = 1.0 / (sw4 * sa3)

    def wpack(wT, KT, MT, sw, fp8):
        packed = wT.reshape(KT, P, MT, P).transpose(1, 2, 0, 3)
        if fp8:
            return _e4(packed * sw)
        return _bf16(packed)

    w1p = np.zeros((P, 2048), np.float32)
    w1p[:4] = W1.T
    shared = {
        "w1": _bf16(w1p * sa1),
        "w2": wpack(W2.T, 16, 16, sw2, "l2" in FP8_LAYERS),
        "w3": wpack(W3T, 16, 16, sw3, "l3" in FP8_LAYERS),
        "w41": wpack(W41.T, 8, 8, sw4, "l4" in FP8_LAYERS),
        "w42": wpack(W42.T, 8, 8, sw4, "l4" in FP8_LAYERS),
        "w51": pack3(W51.T, 8),
        "w52": pack3(W52.T, 8),
        "esc2": _f32(np.full((P, 1), esc2)),
        "esc3": _f32(np.full((P, 1), esc3)),
        "esc4": _f32(np.full((P, 1), esc4)),
        "b1": _f32(b1.reshape(16, P).T * sa1),
        "b2": _f32(b2.reshape(16, P).T * sa2),
        "b3": _f32(b3.reshape(16, P).T * sa3),
        "b41": _f32(b41.reshape(8, P).T),
        "b42": _f32(b42.reshape(8, P).T),
        "b51": b51,
        "b52": b52,
        "stdb": _f32(np.tile(std[None, :], (P, 1))),
        "meanb": _f32(np.tile(mean[None, :], (P, 1))),
        "mlb": _f32(np.tile(mean_label[None, :], (P, 1))),
        "islb": _f32(np.tile((1.0 / std_label)[None, :], (P, 1))),
        "wtv": wt,
    }

    in_maps = []
    for i in range(N_CORES):
        xs = x[i * B : (i + 1) * B]  # (2048, 4)
        m = dict(shared)
        xtp = np.zeros((P, B), np.float32)
        xtp[:4] = xs.T
        m["xt"] = _bf16(xtp)
        m["xn"] = _f32(xs.reshape(CH, P, 4).transpose(1, 0, 2))
        in_maps.append(m)
    return in_maps


def kernel_core(inputs, trace=False):
    nc = _get_nc()
    in_maps = _prep_inputs(inputs)
    res = run_bass_kernel_spmd(
        nc, in_maps, core_ids=list(range(N_CORES)), trace=trace
    )
    shards = []
    for i in range(N_CORES):
        o = res.results[i]["out"]  # (128, 16, 2)
        shards.append(o.transpose(1, 0, 2).reshape(B, 2))
    return np.concatenate(shards, axis=0).astype(np.float32), res


def kernel(**inputs):
    out, _ = kernel_core(inputs)
    return out


# revision 31
# speedup vs baseline: 2.2156x; 1.0079x over previous
"""Trainium2 Bass kernel for nn_ABNet_U (multi-branch MLP + CBF-QP head).

Data-parallel over batch: 16384 rows -> 8 NeuronCores x 2048 rows, weights
replicated and host-prepped into K-major layouts.  The three large middle
GEMMs (L2/L3/L4) run in fp8-e4m3 with DoubleRow perf mode (2 weights per
PE cell -> half the matmul count); L1 and the small heads stay bf16.  All
GEMMs accumulate in fp32 PSUM with fused scale+bias+activation eviction on
the Scalar/Vector engines (fp8 rescales fold into the eviction scale).
The trig/barrier/QP tail runs in fp32 on the Vector engine with batch on
partitions, emitted early so it executes underneath the GEMM phase.
Measured: rel err 1.40e-2 vs the fp32 reference (gate 2e-2), ~336us/core
at full PE clock.  Set FP8_LAYERS = frozenset() for the bf16-exact
fallback (rel err 9.3e-4, ~612us).
"""

import sys

sys.path.insert(0, "/opt/trn_rl_repo")

import numpy as np
import ml_dtypes

import concourse.bass as bass
import concourse.mybir as mybir
import concourse.tile as tile
from concourse import bacc
from concourse.bass_utils import run_bass_kernel_spmd
from concourse.masks import make_identity

N_CORES = 8
B_GLOBAL = 16384
B = B_GLOBAL // N_CORES  # 2048 rows per core
P = 128
CH = B // P              # 16 batch chunks of 128 (tail layout)
NF = 512                 # matmul free-dim chunk
NB = B // NF             # 4 free chunks
HEADS = 10

AF = mybir.ActivationFunctionType
ALU = mybir.AluOpType
AX = mybir.AxisListType
F32 = mybir.dt.float32
BF16 = mybir.dt.bfloat16
I32 = mybir.dt.int32

TWO_PI = float(2.0 * np.pi)
HALF_PI = float(0.5 * np.pi)

# Layers computed in fp8-e4m3 with DoubleRow (2 weights/PE cell, halves the
# matmul count).  Weights use per-tensor scales; activations use static
# per-layer scales calibrated host-side on a row subsample with 4.7x
# headroom to the e4m3 max.
FP8_LAYERS = frozenset({"l2", "l3", "l4"})  # override via configure()
FP8 = mybir.dt.float8e4

_CACHED_NC = None


def _build(fp8_layers=frozenset()):
    nc = bacc.Bacc(
        "TRN2",
        target_bir_lowering=False,
        debug=False,
        enable_asserts=False,
        num_devices=N_CORES,
    )

    def din(name, shape, dt=F32):
        return nc.dram_tensor(name, list(shape), dt, kind="ExternalInput").ap()

    xt = din("xt", (P, B), BF16)
    xn = din("xn", (P, CH, 4))              # x shard, [p, chunk, feat] fp32
    w1 = din("w1", (P, 2048), BF16)
    w2 = din("w2", (P, 16, 16, P), FP8 if "l2" in fp8_layers else BF16)
    w3 = din("w3", (P, 16, 16, P), FP8 if "l3" in fp8_layers else BF16)
    w41 = din("w41", (P, 8, 8, P), FP8 if "l4" in fp8_layers else BF16)
    w42 = din("w42", (P, 8, 8, P), FP8 if "l4" in fp8_layers else BF16)
    esc2 = din("esc2", (P, 1))
    esc3 = din("esc3", (P, 1))
    esc4 = din("esc4", (P, 1))
    w51 = din("w51", (P, 8, 20), BF16)      # [p, kt, m]
    w52 = din("w52", (P, 8, 11), BF16)
    b1 = din("b1", (P, 16))
    b2 = din("b2", (P, 16))
    b3 = din("b3", (P, 16))
    b41 = din("b41", (P, 8))
    b42 = din("b42", (P, 8))
    b51 = din("b51", (20,))
    b52 = din("b52", (11,))
    stdb = din("stdb", (P, 4))
    meanb = din("meanb", (P, 4))
    mlb = din("mlb", (P, 2))
    islb = din("islb", (P, 2))
    wtv = din("wtv", (10,))
    out = nc.dram_tensor("out", [P, CH, 2], F32, kind="ExternalOutput").ap()

    with tile.TileContext(nc) as tc:
        from contextlib import ExitStack

        with ExitStack() as ctx:
            const = ctx.enter_context(tc.tile_pool(name="const", bufs=1))
            wpool = ctx.enter_context(tc.tile_pool(name="wpool", bufs=3))
            hpool = ctx.enter_context(tc.tile_pool(name="hpool", bufs=2))
            psum = ctx.enter_context(tc.tile_pool(name="psum", bufs=4, space="PSUM"))
            tp = ctx.enter_context(tc.tile_pool(name="tp", bufs=1))

            # ---- L1-critical loads first: keep the PE fed from t=0 ----
            b1t = const.tile([P, 16], F32, tag="b1")
            nc.sync.dma_start(b1t[:], b1)
            xtb = const.tile([P, B], BF16, tag="xtb")
            nc.sync.dma_start(xtb[:], xt)
            w1tb = const.tile([P, 2048], BF16, tag="w1tb")
            nc.sync.dma_start(w1tb[:], w1)

            # prefetch: L2's first weight column gates the L1->L2 handoff;
            # the tiny L5 weight tiles ride the otherwise-idle gpsimd queue.
            wcol2_0 = wpool.tile(
                [P, 16, P], FP8 if "l2" in fp8_layers else BF16,
                tag="wcol", name="wcol2_0",
            )
            nc.sync.dma_start(wcol2_0[:], w2[:, 0])
            w51c = const.tile([P, 8, 20], BF16, tag="w51c")
            nc.gpsimd.dma_start(w51c[:], w51)
            w52c = const.tile([P, 8, 11], BF16, tag="w52c")
            nc.gpsimd.dma_start(w52c[:], w52)

            # PE warm-up: dummy matmuls fill the idle window while the L1
            # input DMAs land, and push the HAM clock gate to 8/8 before the
            # real matmul stream begins.
            wrm = const.tile([P, NF], BF16, tag="wrm")
            nc.vector.memset(wrm[:], 0.0)
            wps = psum.tile([P, 2 * NF], F32, tag="mm", name="wps")
            for _ in range(16):
                nc.tensor.matmul(
                    wps[:, :NF], wrm[:, :P], wrm[:], start=True, stop=True
                )

            # warm the ACT engine's function tables in the startup window so
            # the 1.3us table loads don't land mid-stream later
            tbl = const.tile([1, 2], F32, tag="tbl")
            nc.vector.memset(tbl[:], 0.25)
            nc.scalar.activation(tbl[:, 1:2], tbl[:, 0:1], AF.Relu)
            nc.scalar.activation(tbl[:, 1:2], tbl[:, 0:1], AF.Exp)
            nc.scalar.activation(tbl[:, 1:2], tbl[:, 0:1], AF.Sin)
            nc.scalar.activation(tbl[:, 1:2], tbl[:, 0:1], AF.Sigmoid)
            nc.scalar.activation(tbl[:, 1:2], tbl[:, 0:1], AF.Identity)

            # ---- L1: h1 = relu(W1 @ x^T + b1), K=4 zero-padded to 128 ----
            # One matmul per eviction, so evictions bound this layer: split
            # them across the Scalar and Vector engines.
            h1dt = FP8 if "l2" in fp8_layers else BF16
            h2dt = FP8 if "l3" in fp8_layers else BF16
            h3dt = FP8 if "l4" in fp8_layers else BF16
            h1 = hpool.tile([P, 16, B], h1dt, tag="act", name="h1")
            _l1_flip = [0]

            def emit_l1_group(g, m):
                ps = psum.tile([P, 2 * NF], F32, tag="mm", name="l1ps")
                for half in range(2):
                    n = 2 * g + half
                    nc.tensor.matmul(
                        ps[:, half * NF : (half + 1) * NF],
                        w1tb[:, m * P : (m + 1) * P],
                        xtb[:, n * NF : (n + 1) * NF],
                        start=True,
                        stop=True,
                    )
                dst = h1[:, m, 2 * g * NF : 2 * (g + 1) * NF]
                # ACT evicts ~1.45us vs DVE ~1.78us: 9/16 duty on ACT
                if _l1_flip[0] % 16 in (0, 2, 4, 6, 8, 10, 12, 14, 15):
                    nc.scalar.activation(
                        dst, ps[:], AF.Relu, bias=b1t[:, m : m + 1]
                    )
                else:
                    nc.vector.tensor_scalar(
                        dst, ps[:], b1t[:, m : m + 1], 0.0,
                        op0=ALU.add, op1=ALU.max,
                    )
                _l1_flip[0] += 1
                # (sa1 activation scale is folded into W1/b1 on the host)

            # L1 column-group 0 gates L2's start: emit it alone.  Group 1's
            # evictions hide under the interleaved L2-g0 matmul blocks below.
            for m in range(16):
                emit_l1_group(0, m)

            # ---- remaining constants (emitted after L1 so they never gate it)
            b2t = const.tile([P, 16], F32, tag="b2")
            nc.sync.dma_start(b2t[:], b2)
            b3t = const.tile([P, 16], F32, tag="b3")
            nc.sync.dma_start(b3t[:], b3)
            b41t = const.tile([P, 8], F32, tag="b41")
            nc.sync.dma_start(b41t[:], b41)
            b42t = const.tile([P, 8], F32, tag="b42")
            nc.sync.dma_start(b42t[:], b42)
            b51t = const.tile([20, 1], F32, tag="b51")
            nc.sync.dma_start(b51t[:], b51[:, None])
            b52t = const.tile([11, 1], F32, tag="b52")
            nc.sync.dma_start(b52t[:], b52[:, None])
            stdt = const.tile([P, 4], F32, tag="stdt")
            nc.sync.dma_start(stdt[:], stdb)
            meant = const.tile([P, 4], F32, tag="meant")
            nc.sync.dma_start(meant[:], meanb)
            mlt = const.tile([P, 2], F32, tag="mlt")
            nc.sync.dma_start(mlt[:], mlb)
            islt = const.tile([P, 2], F32, tag="islt")
            nc.sync.dma_start(islt[:], islb)
            esc2t = const.tile([P, 1], F32, tag="esc2t")
            nc.sync.dma_start(esc2t[:], esc2)
            esc3t = const.tile([P, 1], F32, tag="esc3t")
            nc.sync.dma_start(esc3t[:], esc3)
            esc4t = const.tile([P, 1], F32, tag="esc4t")
            nc.sync.dma_start(esc4t[:], esc4)
            halfpi = const.tile([P, 1], F32, tag="halfpi")
            nc.vector.memset(halfpi[:], HALF_PI)
            ident = const.tile([P, P], F32)
            make_identity(nc, ident[:])

            # softmax(wt) DVE chain (PE broadcast deferred until after L4)
            wtt = const.tile([1, 10], F32, tag="wtt")
            nc.sync.dma_start(wtt[:], wtv[None, :])
            mx = const.tile([1, 1], F32, tag="mx")
            nc.vector.reduce_max(mx[:, 0:1], wtt[:], axis=AX.X)
            nm = const.tile([1, 1], F32, tag="nm")
            nc.vector.tensor_scalar_mul(nm[:], mx[:], -1.0)
            ex = const.tile([1, 10], F32, tag="ex")
            nc.scalar.activation(ex[:], wtt[:], AF.Exp, bias=nm[:])
            sm = const.tile([1, 1], F32, tag="sm")
            nc.vector.reduce_sum(sm[:, 0:1], ex[:], axis=AX.X)
            inv = const.tile([1, 1], F32, tag="inv")
            nc.vector.reciprocal(inv[:], sm[:])
            wv10 = const.tile([1, 10], F32, tag="wv10")
            nc.vector.tensor_scalar_mul(wv10[:], ex[:], inv[:])
            wvp = const.tile([32, 32], F32, tag="wvp")
            nc.vector.memset(wvp[:], 0.0)
            nc.vector.tensor_copy(
                wvp[0:1, 0:20].rearrange("p (h c) -> p h c", c=2),
                wv10[:, :, None].to_broadcast([1, 10, 2]),
            )
            onesp = const.tile([32, P], F32, tag="onesp")
            nc.vector.memset(onesp[:], 0.0)
            nc.vector.memset(onesp[0:1, :], 1.0)

            # combined x51/x52 head tile (rows 0..19 = x51, 20..30 = x52)
            x5cat = tp.tile([64, B], F32, tag="x5cat")
            nc.vector.memset(x5cat[:], 0.0)

            # ---- tail part 1: geometry from x only — emitted now so the
            # Vector engine computes it underneath the L2..L5 GEMMs.
            def t3(tag, d=1):
                return tp.tile([P, CH, d], F32, tag=tag, name=tag)

            xnt = t3("xnt", 4)
            nc.sync.dma_start(xnt[:], xn)
            x0 = t3("x0", 4)
            nc.vector.tensor_mul(
                x0[:], xnt[:], stdt[:, None, :].to_broadcast([P, CH, 4])
            )
            nc.vector.tensor_add(
                x0[:], x0[:], meant[:, None, :].to_broadcast([P, CH, 4])
            )

            th = x0[:, :, 0::2]   # [P, CH, 2] angles
            wv_ = x0[:, :, 1::2]  # [P, CH, 2] angular velocities

            # range-reduce th -> rs in [-pi, pi]:  rs = th - 2pi*rint(th/2pi)
            q = t3("q", 2)
            qi = tp.tile([P, CH, 2], I32, tag="qi")
            qr = t3("qr", 2)
            rs = t3("rs", 2)
            nc.vector.tensor_scalar_mul(q[:], th, 1.0 / TWO_PI)
            nc.vector.tensor_copy(qi[:], q[:])
            nc.vector.tensor_copy(qr[:], qi[:])
            nc.vector.scalar_tensor_tensor(
                rs[:], in0=qr[:], scalar=-TWO_PI, in1=th,
                op0=ALU.mult, op1=ALU.add,
            )
            # range-reduce th + pi/2 -> rc (for cos)
            qc = t3("qc", 2)
            qci = tp.tile([P, CH, 2], I32, tag="qci")
            qcr = t3("qcr", 2)
            rc = t3("rc", 2)
            nc.vector.tensor_scalar(
                qc[:], th, 1.0 / TWO_PI, 0.25, op0=ALU.mult, op1=ALU.add
            )
            nc.vector.tensor_copy(qci[:], qc[:])
            nc.vector.tensor_copy(qcr[:], qci[:])
            nc.vector.scalar_tensor_tensor(
                rc[:], in0=qcr[:], scalar=-TWO_PI, in1=th,
                op0=ALU.mult, op1=ALU.add,
            )
            nc.vector.tensor_scalar_add(rc[:], rc[:], HALF_PI)

            sn = t3("sn", 2)
            cs = t3("cs", 2)
            nc.scalar.activation(sn[:], rs[:], AF.Sin)
            nc.scalar.activation(cs[:], rc[:], AF.Sin)

            s1, s2 = sn[:, :, 0:1], sn[:, :, 1:2]
            c1, c2 = cs[:, :, 0:1], cs[:, :, 1:2]
            w1v, w2v = wv_[:, :, 0:1], wv_[:, :, 1:2]

            px = t3("px")
            nc.vector.tensor_add(px[:], c1, c2)
            nc.vector.tensor_scalar_mul(px[:], px[:], 3.0)
            py = t3("py")
            nc.vector.tensor_add(py[:], s1, s2)
            nc.vector.tensor_scalar(py[:], py[:], 3.0, -7.0, op0=ALU.mult, op1=ALU.add)

            s1w = t3("s1w")
            nc.vector.tensor_mul(s1w[:], s1, w1v)
            s2w = t3("s2w")
            nc.vector.tensor_mul(s2w[:], s2, w2v)
            vx = t3("vx")
            nc.vector.tensor_add(vx[:], s1w[:], s2w[:])
            nc.vector.tensor_scalar_mul(vx[:], vx[:], -3.0)
            c1w = t3("c1w")
            nc.vector.tensor_mul(c1w[:], c1, w1v)
            c2w = t3("c2w")
            nc.vector.tensor_mul(c2w[:], c2, w2v)
            vy = t3("vy")
            nc.vector.tensor_add(vy[:], c1w[:], c2w[:])
            nc.vector.tensor_scalar_mul(vy[:], vy[:], 3.0)

            pxx = t3("pxx")
            nc.vector.tensor_mul(pxx[:], px[:], px[:])
            pyy = t3("pyy")
            nc.vector.tensor_mul(pyy[:], py[:], py[:])
            # barrier scaled by 16 = alpha*beta scale (4*sigmoid each)
            barrier = t3("barrier")
            nc.vector.tensor_add(barrier[:], pxx[:], pyy[:])
            nc.vector.tensor_scalar(
                barrier[:], barrier[:], 16.0, -256.0, op0=ALU.mult, op1=ALU.add
            )

            pv1 = t3("pv1")
            nc.vector.tensor_mul(pv1[:], px[:], vx[:])
            pv2 = t3("pv2")
            nc.vector.tensor_mul(pv2[:], py[:], vy[:])
            b_dot = t3("b_dot")
            nc.vector.tensor_add(b_dot[:], pv1[:], pv2[:])
            # 2 (from b_dot) * 4 (alpha+beta sigmoid scale)
            nc.vector.tensor_scalar_mul(b_dot[:], b_dot[:], 8.0)

            w1sq = t3("w1sq")
            nc.vector.tensor_mul(w1sq[:], w1v, w1v)
            w2sq = t3("w2sq")
            nc.vector.tensor_mul(w2sq[:], w2v, w2v)
            ca = t3("ca")
            nc.vector.tensor_mul(ca[:], c1, w1sq[:])
            cb = t3("cb")
            nc.vector.tensor_mul(cb[:], c2, w2sq[:])
            nc.vector.tensor_add(ca[:], ca[:], cb[:])   # c1*w1^2 + c2*w2^2
            sa = t3("sa")
            nc.vector.tensor_mul(sa[:], s1, w1sq[:])
            sb = t3("sb")
            nc.vector.tensor_mul(sb[:], s2, w2sq[:])
            nc.vector.tensor_add(sa[:], sa[:], sb[:])   # s1*w1^2 + s2*w2^2

            vxx = t3("vxx")
            nc.vector.tensor_mul(vxx[:], vx[:], vx[:])
            vyy = t3("vyy")
            nc.vector.tensor_mul(vyy[:], vy[:], vy[:])
            vsum = t3("vsum")
            nc.vector.tensor_add(vsum[:], vxx[:], vyy[:])
            nc.vector.tensor_scalar_mul(vsum[:], vsum[:], 2.0)  # 2vx^2+2vy^2

            pca = t3("pca")
            nc.vector.tensor_mul(pca[:], px[:], ca[:])
            psa = t3("psa")
            nc.vector.tensor_mul(psa[:], py[:], sa[:])
            nc.vector.tensor_add(pca[:], pca[:], psa[:])
            lf2b = t3("lf2b")
            nc.vector.scalar_tensor_tensor(
                lf2b[:], in0=pca[:], scalar=-6.0, in1=vsum[:],
                op0=ALU.mult, op1=ALU.add,
            )  # Lf2b = 2(vx^2+vy^2) - 6*(px*ca + py*sa)

            g1 = t3("g1")
            m1 = t3("m1")
            nc.vector.tensor_mul(m1[:], py[:], c1)
            m2 = t3("m2")
            nc.vector.tensor_mul(m2[:], px[:], s1)
            nc.vector.tensor_sub(g1[:], m1[:], m2[:])
            nc.vector.tensor_scalar_mul(g1[:], g1[:], 6.0)
            g2 = t3("g2")
            nc.vector.tensor_mul(m1[:], py[:], c2)
            nc.vector.tensor_mul(m2[:], px[:], s2)
            nc.vector.tensor_sub(g2[:], m1[:], m2[:])
            nc.vector.tensor_scalar_mul(g2[:], g2[:], 6.0)

            gdot = t3("gdot")
            g1sq = t3("g1sq")
            nc.vector.tensor_mul(g1sq[:], g1[:], g1[:])
            g2sq = t3("g2sq")
            nc.vector.tensor_mul(g2sq[:], g2[:], g2[:])
            nc.vector.tensor_add(gdot[:], g1sq[:], g2sq[:])
            igdot = t3("igdot")
            nc.vector.reciprocal(igdot[:], gdot[:])

            # ---- generic streamed GEMM layer ----
            def mlp_layer(wdram, KT, MT, MD, hin, kin_base, btile, evict,
                          prefetched=None, dr=False):
                wdt = FP8 if dr else BF16
                for m in range(MT):
                    mp = min(P, MD - m * P)
                    if prefetched is not None and m in prefetched:
                        wcol = prefetched[m]
                    else:
                        wcol = wpool.tile([P, KT, mp], wdt, tag="wcol")
                        if len(wdram.shape) == 4:
                            nc.sync.dma_start(wcol[:], wdram[:, m])
                        else:
                            nc.sync.dma_start(wcol[:], wdram)
                    for g in range(NB // 2):
                        ps = psum.tile([P, 2 * NF], F32, tag="mm")
                        for half in range(2):
                            n = 2 * g + half
                            if dr:
                                for k2 in range(KT // 2):
                                    nc.tensor.matmul(
                                        ps[:mp, half * NF : (half + 1) * NF],
                                        wcol[:, 2 * k2 : 2 * k2 + 2, :],
                                        hin[:, kin_base + 2 * k2 : kin_base + 2 * k2 + 2,
                                            n * NF : (n + 1) * NF],
                                        start=(k2 == 0),
                                        stop=(k2 == KT // 2 - 1),
                                        perf_mode=mybir.MatmulPerfMode.DoubleRow,
                                    )
                            else:
                                for k in range(KT):
                                    nc.tensor.matmul(
                                        ps[:mp, half * NF : (half + 1) * NF],
                                        wcol[:, k, :],
                                        hin[:, kin_base + k, n * NF : (n + 1) * NF],
                                        start=(k == 0),
                                        stop=(k == KT - 1),
                                    )
                        evict(m, g, ps[:mp])

            # ---- L2 / L3 / L4 ----
            h2 = hpool.tile([P, 16, B], h2dt, tag="act", name="h2")

            def ev_h(hout, btile, m_off=0, scale=1.0):
                def _e(m, g, ps):
                    nc.scalar.activation(
                        hout[:, m_off + m, 2 * g * NF : 2 * (g + 1) * NF], ps,
                        AF.Relu, bias=btile[:, m : m + 1], scale=scale,
                    )
                return _e

            sc2 = esc2t[:]
            ev2 = ev_h(h2, b2t, 0, sc2)
            l2dr = "l2" in fp8_layers
            w2dt = FP8 if l2dr else BF16

            def emit_l2_group(m, g, wcol):
                ps = psum.tile([P, 2 * NF], F32, tag="mm", name="l2ps")
                for half in range(2):
                    n = 2 * g + half
                    if l2dr:
                        for k2 in range(8):
                            nc.tensor.matmul(
                                ps[:, half * NF : (half + 1) * NF],
                                wcol[:, 2 * k2 : 2 * k2 + 2, :],
                                h1[:, 2 * k2 : 2 * k2 + 2, n * NF : (n + 1) * NF],
                                start=(k2 == 0),
                                stop=(k2 == 7),
                                perf_mode=mybir.MatmulPerfMode.DoubleRow,
                            )
                    else:
                        for k in range(16):
                            nc.tensor.matmul(
                                ps[:, half * NF : (half + 1) * NF],
                                wcol[:, k, :],
                                h1[:, k, n * NF : (n + 1) * NF],
                                start=(k == 0),
                                stop=(k == 15),
                            )
                ev2(m, g, ps[:])

            # Pass A: L1 group-1 evictions hide under the L2 group-0 blocks.
            for m in range(16):
                emit_l1_group(1, m)
                if m == 0:
                    wcol = wcol2_0
                else:
                    wcol = wpool.tile([P, 16, P], w2dt, tag="wcol", name="w2a")
                    nc.sync.dma_start(wcol[:], w2[:, m])
                emit_l2_group(m, 0, wcol)
            # Pass B: second column-group (weights re-streamed; traffic hides).
            for m in range(16):
                wcol = wpool.tile([P, 16, P], w2dt, tag="wcol", name="w2b")
                nc.sync.dma_start(wcol[:], w2[:, m])
                emit_l2_group(m, 1, wcol)

            h3 = hpool.tile([P, 16, B], h3dt, tag="act", name="h3")
            sc3 = esc3t[:]
            mlp_layer(w3, 16, 16, 2048, h2, 0, b3t, ev_h(h3, b3t, 0, sc3),
                      dr="l3" in fp8_layers)

            h4 = hpool.tile([P, 16, B], BF16, tag="act", name="h4")
            sc4 = esc4t[:]
            mlp_layer(w41, 8, 8, 1024, h3, 0, b41t, ev_h(h4, b41t, 0, sc4),
                      dr="l4" in fp8_layers)
            mlp_layer(w42, 8, 8, 1024, h3, 8, b42t, ev_h(h4, b42t, 8, sc4),
                      dr="l4" in fp8_layers)

            # ---- L5 into the combined head tile ----
            def ev_51(m, g, ps):
                nc.vector.tensor_scalar_add(
                    x5cat[:20, 2 * g * NF : 2 * (g + 1) * NF], ps, b51t[:]
                )

            def ev_52(m, g, ps):
                nc.scalar.activation(
                    x5cat[32:43, 2 * g * NF : 2 * (g + 1) * NF], ps, AF.Sigmoid,
                    bias=b52t[:],
                )

            mlp_layer(w51, 8, 1, 20, h4, 0, b51t, ev_51, prefetched={0: w51c})
            mlp_layer(w52, 8, 1, 11, h4, 8, b52t, ev_52, prefetched={0: w52c})

            # wv broadcast to all partitions (PE hits this after L4/L5)
            pwv = psum.tile([P, 2 * NF], F32, tag="mm", name="pwv")
            nc.tensor.matmul(pwv[:, :32], onesp[:], wvp[:], start=True, stop=True)
            wv20 = const.tile([P, 20], F32, tag="wv20")
            nc.vector.tensor_copy(wv20[:], pwv[:, :20])

            # ---- transpose the combined head tile: 16 chunks of [32,128] ----
            x5t = tp.tile([P, CH, 43], F32, tag="x5t")
            for c in range(CH):
                pt = psum.tile([P, 2 * NF], F32, tag="mm", name="pt")
                nc.tensor.transpose(
                    pt[:, :64], x5cat[:, c * P : (c + 1) * P], ident[:64, :64]
                )
                if c < 8 and c % 2 == 0:
                    nc.vector.tensor_copy(x5t[:, c, :], pt[:, :43])
                else:
                    nc.scalar.copy(x5t[:, c, :], pt[:, :43])

            # ---- tail part 2: per-head analytic QP ----
            # Two chunk-halves: the first half's vector math overlaps the
            # scalar-engine transpose copies of the second half.
            g12 = tp.tile([P, CH, 2], F32, tag="g12", name="g12")
            nc.vector.tensor_copy(g12[:, :, 0:1], g1[:])
            nc.vector.tensor_copy(g12[:, :, 1:2], g2[:])
            apb = t3("apb", 10)
            ab = t3("ab", 10)
            hv = t3("hv", 10)
            hv2 = t3("hv2", 10)
            refg = tp.tile([P, CH, 10, 2], F32, tag="refg", name="refg")
            viol = t3("viol", 10)
            lam = t3("lam", 10)
            wlam = t3("wlam", 10)
            S = t3("S")
            wref = t3("wref", 20)
            rbxy = t3("rbxy", 2)
            rtxy = t3("rtxy", 2)
            ot = t3("ot", 2)

            for c0, c1 in ((0, CH // 2), (CH // 2, CH)):
                W = c1 - c0
                BW10 = [P, W, 10]
                cs_ = (slice(None), slice(c0, c1))
                alpha = x5t[:, c0:c1, 32:33]
                betas = x5t[:, c0:c1, 33:43]
                nc.vector.tensor_add(apb[*cs_], betas, alpha.to_broadcast(BW10))
                nc.vector.tensor_mul(ab[*cs_], betas, alpha.to_broadcast(BW10))
                nc.vector.tensor_mul(
                    hv[*cs_], apb[*cs_], b_dot[:, c0:c1, :].to_broadcast(BW10)
                )
                nc.vector.tensor_mul(
                    hv2[*cs_], ab[*cs_], barrier[:, c0:c1, :].to_broadcast(BW10)
                )
                nc.vector.tensor_add(hv[*cs_], hv[*cs_], hv2[*cs_])
                nc.vector.tensor_add(
                    hv[*cs_], hv[*cs_], lf2b[:, c0:c1, :].to_broadcast(BW10)
                )
                nc.vector.tensor_mul(
                    refg[:, c0:c1],
                    x5t[:, c0:c1, 0:20].rearrange("p c (h two) -> p c h two", two=2),
                    g12[:, c0:c1, None, :].to_broadcast([P, W, 10, 2]),
                )
                nc.vector.reduce_sum(viol[*cs_, slice(None)], refg[:, c0:c1], axis=AX.X)
                nc.vector.tensor_sub(viol[*cs_], viol[*cs_], hv[*cs_])
                nc.vector.tensor_mul(
                    lam[*cs_], viol[*cs_], igdot[:, c0:c1, :].to_broadcast(BW10)
                )
                nc.vector.tensor_scalar_max(lam[*cs_], lam[*cs_], 0.0)
                nc.vector.tensor_mul(
                    wlam[*cs_], lam[*cs_], wv20[:, None, 0::2].to_broadcast(BW10)
                )
                nc.vector.reduce_sum(S[*cs_, 0], wlam[*cs_], axis=AX.X)
                nc.vector.tensor_mul(
                    wref[*cs_], x5t[:, c0:c1, 0:20],
                    wv20[:, None, :].to_broadcast([P, W, 20]),
                )
                nc.vector.reduce_sum(
                    rbxy[*cs_, slice(None)],
                    wref[*cs_].rearrange("p c (h two) -> p c two h", two=2),
                    axis=AX.X,
                )
                nc.vector.tensor_mul(
                    rtxy[*cs_], g12[:, c0:c1], S[:, c0:c1, :].to_broadcast([P, W, 2])
                )
                nc.vector.tensor_sub(rtxy[*cs_], rtxy[*cs_], rbxy[*cs_])
                nc.vector.tensor_scalar(
                    ot[*cs_, 0], rtxy[*cs_, 0], mlt[:, 0:1], islt[:, 0:1],
                    op0=ALU.subtract, op1=ALU.mult,
                )
                nc.vector.tensor_scalar(
                    ot[*cs_, 1], rtxy[*cs_, 1], mlt[:, 1:2], islt[:, 1:2],
                    op0=ALU.subtract, op1=ALU.mult,
                )

            nc.sync.dma_start(out, ot[:])

    nc.compile()
    return nc


def configure(fp8_layers):
    """Select fp8 layers; must be called before the first kernel() call."""
    global FP8_LAYERS, _CACHED_NC
    if frozenset(fp8_layers) != FP8_LAYERS:
        FP8_LAYERS = frozenset(fp8_layers)
        _CACHED_NC = None


def _get_nc():
    global _CACHED_NC
    if _CACHED_NC is None:
        _CACHED_NC = _build(FP8_LAYERS)
    return _CACHED_NC


def _bf16(a):
    return np.ascontiguousarray(a.astype(ml_dtypes.bfloat16))


def _f32(a):
    return np.ascontiguousarray(np.asarray(a, dtype=np.float32))


def _e4(a):
    dt = mybir.dt.np(mybir.dt.float8e4)
    return np.ascontiguousarray(a.astype(dt))


def _prep_inputs(inputs):
    x = _f32(inputs["x"])
    mean = _f32(inputs["mean"])
    std = _f32(inputs["std"])
    mean_label = _f32(inputs["mean_label"])
    std_label = _f32(inputs["std_label"])
    wt = _f32(inputs["wt"])
    W1, b1 = _f32(inputs["W1"]), _f32(inputs["b1"])
    W2, b2 = _f32(inputs["W2"]), _f32(inputs["b2"])
    W31, b31 = _f32(inputs["W31"]), _f32(inputs["b31"])
    W32, b32 = _f32(inputs["W32"]), _f32(inputs["b32"])
    W41, b41 = _f32(inputs["W41"]), _f32(inputs["b41"])
    W42, b42 = _f32(inputs["W42"]), _f32(inputs["b42"])
    W51, b51 = _f32(inputs["W51"]), _f32(inputs["b51"])
    W52, b52 = _f32(inputs["W52"]), _f32(inputs["b52"])

    def pack4(wT, KT, MT):  # (K, M) -> (128, MT, KT, 128)
        return _bf16(wT.reshape(KT, P, MT, P).transpose(1, 2, 0, 3))

    def pack3(wT, KT):  # (K, M) -> (128, KT, M)
        K, M = wT.shape
        return _bf16(wT.reshape(KT, P, M).transpose(1, 0, 2))

    W3T = np.concatenate([W31.T, W32.T], axis=1)  # (2048, 2048)
    b3 = np.concatenate([b31, b32])

    # fp8 calibration: static activation scales from a row subsample
    # (4.7x headroom to the e4m3 max), per-tensor weight scales.
    sa1 = sa2 = sa3 = 1.0
    sw2 = sw3 = sw4 = 1.0
    if FP8_LAYERS:
        xs = x[:512]
        h1s = np.maximum(xs @ W1.T + b1, 0.0)
        if "l2" in FP8_LAYERS:
            sa1 = 96.0 / max(float(np.abs(h1s).max()), 1e-30)
            sw2 = 192.0 / max(float(np.abs(W2).max()), 1e-30)
        if "l3" in FP8_LAYERS or "l4" in FP8_LAYERS:
            h2s = np.maximum(h1s @ W2.T + b2, 0.0)
            if "l3" in FP8_LAYERS:
                sa2 = 96.0 / max(float(np.abs(h2s).max()), 1e-30)
                sw3 = 192.0 / max(float(np.abs(W3T).max()), 1e-30)
            if "l4" in FP8_LAYERS:
                h3s = np.maximum(h2s @ W3T + b3, 0.0)
                sa3 = 96.0 / max(float(np.abs(h3s).max()), 1e-30)
                sw4 = 192.0 / max(
                    float(max(np.abs(W41).max(), np.abs(W42).max())), 1e-30
                )
    esc2 = sa2 / (sw2 * sa1)
    esc3 = sa3 / (sw3 * sa2)
    esc4 = 1.0 / (sw4 * sa3)

    def wpack(wT, KT, MT, sw, fp8):
        packed = wT.reshape(KT, P, MT, P).transpose(1, 2, 0, 3)
        if fp8:
            return _e4(packed * sw)
        return _bf16(packed)

    w1p = np.zeros((P, 2048), np.float32)
    w1p[:4] = W1.T
    shared = {
        "w1": _bf16(w1p * sa1),
        "w2": wpack(W2.T, 16, 16, sw2, "l2" in FP8_LAYERS),
        "w3": wpack(W3T, 16, 16, sw3, "l3" in FP8_LAYERS),
        "w41": wpack(W41.T, 8, 8, sw4, "l4" in FP8_LAYERS),
        "w42": wpack(W42.T, 8, 8, sw4, "l4" in FP8_LAYERS),
        "w51": pack3(W51.T, 8),
        "w52": pack3(W52.T, 8),
        "esc2": _f32(np.full((P, 1), esc2)),
        "esc3": _f32(np.full((P, 1), esc3)),
        "esc4": _f32(np.full((P, 1), esc4)),
        "b1": _f32(b1.reshape(16, P).T * sa1),
        "b2": _f32(b2.reshape(16, P).T * sa2),
        "b3": _f32(b3.reshape(16, P).T * sa3),
        "b41": _f32(b41.reshape(8, P).T),
        "b42": _f32(b42.reshape(8, P).T),
        "b51": b51,
        "b52": b52,
        "stdb": _f32(np.tile(std[None, :], (P, 1))),
        "meanb": _f32(np.tile(mean[None, :], (P, 1))),
        "mlb": _f32(np.tile(mean_label[None, :], (P, 1))),
        "islb": _f32(np.tile((1.0 / std_label)[None, :], (P, 1))),
        "wtv": wt,
    }

    in_maps = []
    for i in range(N_CORES):
        xs = x[i * B : (i + 1) * B]  # (2048, 4)
        m = dict(shared)
        xtp = np.zeros((P, B), np.float32)
        xtp[:4] = xs.T
        m["xt"] = _bf16(xtp)
        m["xn"] = _f32(xs.reshape(CH, P, 4).transpose(1, 0, 2))
        in_maps.append(m)
    return in_maps


def kernel_core(inputs, trace=False):
    nc = _get_nc()
    in_maps = _prep_inputs(inputs)
    res = run_bass_kernel_spmd(
        nc, in_maps, core_ids=list(range(N_CORES)), trace=trace
    )
    shards = []
    for i in range(N_CORES):
        o = res.results[i]["out"]  # (128, 16, 2)
        shards.append(o.transpose(1, 0, 2).reshape(B, 2))
    return np.concatenate(shards, axis=0).astype(np.float32), res


def kernel(**inputs):
    out, _ = kernel_core(inputs)
    return out
